# revision 1
# baseline (speedup 1.0000x reference)
"""Cross-modal attention TRN2 kernel.

Problem: B=4, N=2048, IN_DIM=DIM=1024, HEADS=8, D_HEAD=128, scale=DIM**-0.5.
  q = x_a @ W_q.T ; k,v = split(x_b @ W_kv.T) ; per-head softmax(q k^T/32) v ;
  out = merge_heads @ W_out.T + b_out

Sharding over 8 cores: core c -> batch b=c//2, head-half hh=c%2 (4 heads,
512 of DIM).  W_q/W_kv column-sharded, W_out row-sharded (Megatron); each
core emits a partial output projection y_cT = (W_out[:, slice] @ O_half)
of shape [DIM, N]; host sums the two head-half partials per batch, adds
b_out, transposes back.

Device layout: everything transposed ([feature, token]) so all matmuls
contract over the partition dim.  Host feeds x^T and W^T (cheap numpy
prep); device does:
  phase 1: Q^T = WqT.T @ xaT, K^T likewise, V (natural [j, dv])
  phase 2: per (head, 1024-token block): dots^T = K_tile^T.T... i.e.
           s^T[j,i] = sum_d K^T[d,j] Q^T[d,i]; exp on ACT (no max
           subtraction -- |s*scale| < ~1 by construction of the problem
           scale); PV and a ones-row matmul (denominator) accumulate over
           j-tiles in PSUM; normalize with reciprocal broadcast.
  phase 3: y^T = WoT.T @ O^T, DMA PSUM->DRAM.
All matmuls run as float32r (full PE rate at N=512).
"""

import numpy as np

B, N, IN_DIM, DIM, HEADS = 4, 2048, 1024, 1024, 8
D_HEAD = DIM // HEADS          # 128
SCALE = DIM ** -0.5            # 1/32
NCORES = 8
HH = HEADS // 2                # 4 heads per core
DVC = HH * D_HEAD              # 512 dv per core
P = 128
KT = IN_DIM // P               # 8 contraction tiles
NJT = N // P                   # 16 j tiles
NIB = N // 512                 # 4 i-blocks of 512
IB2 = N // 1024                # 2 i-blocks of 1024

_TRACE = False
REPS = 1
LAST_EXEC_NS = None
LAST_RESULTS = None
_nc_cache = []


def _build_nc(reps=1):
    import concourse.tile as tile
    from concourse import bacc, mybir

    f32 = mybir.dt.float32
    f32r = mybir.dt.float32r
    Exp = mybir.ActivationFunctionType.Exp

    nc = bacc.Bacc("TRN2", debug=False, num_devices=NCORES)

    xaT = nc.dram_tensor("xaT", [IN_DIM, N], f32r, kind="ExternalInput").ap()
    xbT = nc.dram_tensor("xbT", [IN_DIM, N], f32r, kind="ExternalInput").ap()
    wqT = nc.dram_tensor("wqT", [IN_DIM, DVC], f32r, kind="ExternalInput").ap()
    wkT = nc.dram_tensor("wkT", [IN_DIM, DVC], f32r, kind="ExternalInput").ap()
    wvT = nc.dram_tensor("wvT", [IN_DIM, DVC], f32r, kind="ExternalInput").ap()
    woT = nc.dram_tensor("woT", [DVC, DIM], f32r, kind="ExternalInput").ap()
    ones_d = nc.dram_tensor("ones", [P, 1], f32r, kind="ExternalInput").ap()
    yT = nc.dram_tensor("yT", [DIM, N], f32, kind="ExternalOutput").ap()

    with tile.TileContext(nc) as tc:
      for _rep in range(reps):
        with tc.tile_pool(name="persist", bufs=1) as persist:
            qT_sb = persist.tile([P, HH, N], f32r)      # [d%128, head, i]
            kT_sb = persist.tile([P, HH, N], f32r)      # [d%128, head, j]
            v_sb = persist.tile([P, NJT, DVC], f32r)    # [j%128, jt, dv]
            oT_ts = [[persist.tile([P, 1024], f32r, tag=f"o{h}_{bb}",
                                   name=f"o{h}_{bb}")
                      for bb in range(IB2)] for h in range(HH)]
            ones_sb = persist.tile([P, 1], f32r)
            nc.sync.dma_start(out=ones_sb, in_=ones_d)

            # ---------------- phase 1: projections ----------------
            BW = 256  # streaming block width (>=256 keeps f32r full rate)
            NB = N // BW
            with tc.tile_pool(name="wpool", bufs=1) as wpool, \
                 tc.tile_pool(name="xblk", bufs=3) as xblk, \
                 tc.tile_pool(name="psum1", bufs=4, space="PSUM") as psum1:
                wq_ts = [wpool.tile([P, DVC], f32r, tag=f"wq{kt}", name=f"wq{kt}")
                         for kt in range(KT)]
                wk_ts = [wpool.tile([P, DVC], f32r, tag=f"wk{kt}", name=f"wk{kt}")
                         for kt in range(KT)]
                wv_ts = [wpool.tile([P, DVC], f32r, tag=f"wv{kt}", name=f"wv{kt}")
                         for kt in range(KT)]

                for ib in range(NB):
                    xa_blk = xblk.tile([P, KT, BW], f32r, tag="xblk")
                    nc.sync.dma_start(
                        out=xa_blk,
                        in_=xaT[:, ib * BW:(ib + 1) * BW]
                        .rearrange("(kt p) i -> p kt i", p=P))
                    if ib == 0:
                        # after the first x block so the first matmul's
                        # operands land earliest in DMA queue order
                        for kt in range(KT):
                            nc.sync.dma_start(
                                out=wq_ts[kt],
                                in_=wqT[kt * P:(kt + 1) * P, :])
                    for dt in range(HH):
                        ps = psum1.tile([P, BW], f32, tag="ps1")
                        for kt in range(KT):
                            nc.tensor.matmul(
                                ps,
                                wq_ts[kt][:, dt * P:(dt + 1) * P],
                                xa_blk[:, kt, :],
                                start=(kt == 0), stop=(kt == KT - 1))
                        nc.vector.tensor_copy(
                            qT_sb[:, dt, ib * BW:(ib + 1) * BW], ps)

                for jb in range(NB):
                    xb_blk = xblk.tile([P, KT, BW], f32r, tag="xblk")
                    nc.sync.dma_start(
                        out=xb_blk,
                        in_=xbT[:, jb * BW:(jb + 1) * BW]
                        .rearrange("(kt p) i -> p kt i", p=P))
                    if jb == 0:
                        for kt in range(KT):
                            nc.sync.dma_start(
                                out=wk_ts[kt],
                                in_=wkT[kt * P:(kt + 1) * P, :])
                            nc.sync.dma_start(
                                out=wv_ts[kt],
                                in_=wvT[kt * P:(kt + 1) * P, :])
                    for dt in range(HH):
                        ps = psum1.tile([P, BW], f32, tag="ps1")
                        for kt in range(KT):
                            nc.tensor.matmul(
                                ps,
                                wk_ts[kt][:, dt * P:(dt + 1) * P],
                                xb_blk[:, kt, :],
                                start=(kt == 0), stop=(kt == KT - 1))
                        nc.vector.tensor_copy(
                            kT_sb[:, dt, jb * BW:(jb + 1) * BW], ps)
                    for j2 in range(BW // P):
                        jt = jb * (BW // P) + j2
                        ps = psum1.tile([P, DVC], f32, tag="psv")
                        for kt in range(KT):
                            nc.tensor.matmul(
                                ps,
                                xb_blk[:, kt, j2 * P:(j2 + 1) * P],
                                wv_ts[kt],
                                start=(kt == 0), stop=(kt == KT - 1))
                        nc.vector.tensor_copy(v_sb[:, jt, :], ps)

            # ---------------- phase 2: attention ----------------
            with tc.tile_pool(name="expp", bufs=8) as expp, \
                 tc.tile_pool(name="bcp", bufs=2) as bcp, \
                 tc.tile_pool(name="rcp", bufs=2) as rcp, \
                 tc.tile_pool(name="dotsp", bufs=2, space="PSUM") as dotsp, \
                 tc.tile_pool(name="avp", bufs=1, space="PSUM") as avp, \
                 tc.tile_pool(name="denp", bufs=1, space="PSUM") as denp:
                LAG = 2   # PV/ones trail dots/exp by 2 j-tiles so the PE
                          # never waits on the ACT exp of the current tile
                for ib in range(IB2):
                    for h in range(HH):
                        i0 = ib * 1024
                        po = avp.tile([P, 1024], f32)
                        pd = denp.tile([1, 1024], f32)
                        ets = {}
                        for jt in range(NJT + LAG):
                            if jt < NJT:
                                ps = dotsp.tile([P, 1024], f32, tag="ps")
                                k_l = kT_sb[:, h, jt * P:(jt + 1) * P]
                                for hf in range(2):
                                    nc.tensor.matmul(
                                        ps[:, hf * 512:(hf + 1) * 512],
                                        k_l,
                                        qT_sb[:, h,
                                              i0 + hf * 512:i0 + (hf + 1) * 512],
                                        start=True, stop=True)
                                et = expp.tile([P, 1024], f32r, tag="exp")
                                nc.scalar.activation(et, ps, Exp, scale=SCALE)
                                ets[jt] = et
                            if jt >= LAG:
                                jd = jt - LAG
                                et = ets.pop(jd)
                                v_l = v_sb[:, jd, h * P:(h + 1) * P]
                                for hf in range(2):
                                    sl = slice(hf * 512, (hf + 1) * 512)
                                    nc.tensor.matmul(
                                        po[:, sl], v_l, et[:, sl],
                                        start=(jd == 0), stop=(jd == NJT - 1))
                                    nc.tensor.matmul(
                                        pd[:, sl], ones_sb,
                                        et[:, sl],
                                        start=(jd == 0), stop=(jd == NJT - 1))
                        # drain the PV accumulator to SBUF right away so the
                        # PSUM bank frees for the next block; normalize there.
                        osl = oT_ts[h][ib]
                        nc.vector.tensor_copy(osl, po)
                        rc = rcp.tile([1, 1024], f32, tag="rc")
                        nc.vector.reciprocal(rc, pd)
                        bc = bcp.tile([P, 1024], f32, tag="bc")
                        nc.gpsimd.partition_broadcast(bc, rc)
                        nc.vector.tensor_mul(osl, osl, bc)

                # ---------------- phase 3: output projection ----------------
                # y-psum tiles share the dots pool slots (tag "ps"), which
                # free as the exp of the final j-tiles completes -- a fresh
                # PSUM pool would wait on the whole attention stack instead.
                with tc.tile_pool(name="wop", bufs=1) as wop, \
                     tc.tile_pool(name="ystage", bufs=4) as ystage:
                    wo_sb = wop.tile([P, HH, DIM], f32r)
                    nc.sync.dma_start(
                        out=wo_sb, in_=woT.rearrange("(dt p) e -> p dt e", p=P))
                    for ib in range(NIB):
                        bb, half = divmod(ib, 2)
                        for e8 in range(DIM // P):
                            ps = dotsp.tile([P, 512], f32, tag="ps")
                            for dt in range(HH):
                                nc.tensor.matmul(
                                    ps,
                                    wo_sb[:, dt, e8 * P:(e8 + 1) * P],
                                    oT_ts[dt][bb][:, half * 512:(half + 1) * 512],
                                    start=(dt == 0), stop=(dt == HH - 1))
                            ys = ystage.tile([P, 512], f32, tag="ys")
                            if ib % 2 == 0:
                                nc.vector.tensor_copy(ys, ps)
                            else:
                                nc.scalar.copy(ys, ps)
                            nc.sync.dma_start(
                                out=yT[e8 * P:(e8 + 1) * P,
                                       ib * 512:(ib + 1) * 512],
                                in_=ys)

    nc.compile()
    return nc


_nc_by_reps = {}


def _get_nc(reps=1):
    if reps not in _nc_by_reps:
        _nc_by_reps[reps] = _build_nc(reps)
    return _nc_by_reps[reps]


def kernel(x_a, x_b, W_q, W_kv, W_out, b_out):
    global LAST_EXEC_NS, LAST_RESULTS
    from concourse import bass_utils

    x_a = np.asarray(x_a, dtype=np.float32)
    x_b = np.asarray(x_b, dtype=np.float32)
    W_q = np.asarray(W_q, dtype=np.float32)
    W_kv = np.asarray(W_kv, dtype=np.float32)
    W_out = np.asarray(W_out, dtype=np.float32)
    b_out = np.asarray(b_out, dtype=np.float32)

    nc = _get_nc(REPS)

    xaT = [np.ascontiguousarray(x_a[b].T) for b in range(B)]
    xbT = [np.ascontiguousarray(x_b[b].T) for b in range(B)]
    in_maps = []
    for c in range(NCORES):
        b, hh = divmod(c, 2)
        hs = hh * DVC
        in_maps.append({
            "xaT": xaT[b],
            "xbT": xbT[b],
            "wqT": np.ascontiguousarray(W_q[hs:hs + DVC].T),
            "wkT": np.ascontiguousarray(W_kv[hs:hs + DVC].T),
            "wvT": np.ascontiguousarray(W_kv[DIM + hs:DIM + hs + DVC].T),
            "woT": np.ascontiguousarray(W_out[:, hs:hs + DVC].T),
            "ones": np.ones((P, 1), dtype=np.float32),
        })

    res = bass_utils.run_bass_kernel_spmd(
        nc, in_maps, core_ids=list(range(NCORES)), trace=_TRACE)
    LAST_EXEC_NS = res.exec_time_ns
    LAST_RESULTS = res

    out = np.empty((B, N, DIM), dtype=np.float32)
    for b in range(B):
        acc = res.results[2 * b]["yT"] + res.results[2 * b + 1]["yT"]
        out[b] = acc.T + b_out
    return out


def _make_in_maps(x_a, x_b, W_q, W_kv, W_out):
    xaT = [np.ascontiguousarray(x_a[b].T) for b in range(B)]
    xbT = [np.ascontiguousarray(x_b[b].T) for b in range(B)]
    in_maps = []
    for c in range(NCORES):
        b, hh = divmod(c, 2)
        hs = hh * DVC
        in_maps.append({
            "xaT": xaT[b],
            "xbT": xbT[b],
            "wqT": np.ascontiguousarray(W_q[hs:hs + DVC].T),
            "wkT": np.ascontiguousarray(W_kv[hs:hs + DVC].T),
            "wvT": np.ascontiguousarray(W_kv[DIM + hs:DIM + hs + DVC].T),
            "woT": np.ascontiguousarray(W_out[:, hs:hs + DVC].T),
            "ones": np.ones((P, 1), dtype=np.float32),
        })
    return in_maps


def bench(inputs, reps_pair=(1, 9), iters=5):
    """Measure on-device time per kernel body via rep-delta wall timing."""
    import time
    from concourse import bass_utils
    ins = {k: np.asarray(v, dtype=np.float32) for k, v in inputs.items()
           if k != "b_out"}
    in_maps = _make_in_maps(ins["x_a"], ins["x_b"], ins["W_q"], ins["W_kv"],
                            ins["W_out"])
    walls = {}
    for reps in reps_pair:
        nc = _get_nc(reps)
        # warm-up (compile+cache)
        bass_utils.run_bass_kernel_spmd(nc, in_maps, core_ids=list(range(NCORES)))
        ts = []
        for _ in range(iters):
            t0 = time.perf_counter()
            bass_utils.run_bass_kernel_spmd(nc, in_maps,
                                            core_ids=list(range(NCORES)))
            ts.append(time.perf_counter() - t0)
        walls[reps] = min(ts)
        print(f"reps={reps}: wall min={walls[reps]*1e3:.2f} ms  all={[f'{t*1e3:.1f}' for t in ts]}")
    r0, r1 = reps_pair
    ns = (walls[r1] - walls[r0]) / (r1 - r0) * 1e9
    print(f"per-body device time: {ns:.0f} ns")
    return ns



# revision 2
# speedup vs baseline: 1.2415x; 1.2415x over previous
"""Cross-modal attention TRN2 kernel (bf16 pipeline).

Problem: B=4, N=2048, IN_DIM=DIM=1024, HEADS=8, D_HEAD=128, scale=DIM**-0.5.
  q = x_a @ W_q.T ; k,v = split(x_b @ W_kv.T) ; per-head softmax(q k^T/32) v ;
  out = merge_heads @ W_out.T + b_out

Sharding over 8 cores: core c -> batch b=c//2, head-half hh=c%2 (4 heads,
512 of DIM).  W_q/W_kv column-sharded, W_out row-sharded (Megatron); each
core emits a partial output projection y_cT = (W_out[:, slice] @ O_half)
of shape [DIM, N]; host sums the two head-half partials per batch, adds
b_out, transposes back.

Device layout: everything transposed ([feature, token]) so all matmuls
contract over the partition dim.  All matmul operands are bf16 (host
casts x and W), PSUM accumulation f32; measured end-to-end rel-err vs the
f32 reference ~5e-3 (tolerance 2e-2).

  phase 1: K^T/V from x_b blocks (shared SBUF block), Q^T from x_a;
           512-wide moving blocks, one [128,512] psum per (dt|jt) with
           8 contraction matmuls.  K/Q psum->SBUF copies on DVE, V on ACT.
  phase 2: per (1024-i-block, head): for each of 16 j-tiles: dots^T
           (2 matmuls), exp on ACT (no max subtraction; |s*scale| < ~1),
           and the softmax denominator accumulated OFF the PE: DVE sums
           even j-tiles into accV, GpSimd odd into accG (elementwise adds
           of the exp tiles).  PV trails by LAG j-tiles in a second psum.
           Epilogue: 4 narrow reduce-matmuls (all-ones stationary) give
           den as a [128,1024] psum (rows identical), DVE
           reciprocal_approx_fast -> rc, one fused DVE multiply
           osl = po * rc drains PV psum.
  phase 3: y^T = WoT.T @ O^T (512-wide), staging copies alternate
           DVE/ACT, DMA -> DRAM f32.
"""

import numpy as np

B, N, IN_DIM, DIM, HEADS = 4, 2048, 1024, 1024, 8
D_HEAD = DIM // HEADS          # 128
SCALE = DIM ** -0.5            # 1/32
NCORES = 8
HH = HEADS // 2                # 4 heads per core
DVC = HH * D_HEAD              # 512 dv per core
P = 128
KT = IN_DIM // P               # 8 contraction tiles
NJT = N // P                   # 16 j tiles
NIB = N // 512                 # 4 i-blocks of 512
IB2 = N // 1024                # 2 i-blocks of 1024

_TRACE = False
REPS = 1
LAST_EXEC_NS = None
LAST_RESULTS = None


def _build_nc(reps=1):
    import concourse.tile as tile
    from concourse import bacc, mybir

    f32 = mybir.dt.float32
    bf16 = mybir.dt.bfloat16
    Exp = mybir.ActivationFunctionType.Exp

    nc = bacc.Bacc("TRN2", debug=False, num_devices=NCORES)

    xaT = nc.dram_tensor("xaT", [IN_DIM, N], bf16, kind="ExternalInput").ap()
    xbT = nc.dram_tensor("xbT", [IN_DIM, N], bf16, kind="ExternalInput").ap()
    wqT = nc.dram_tensor("wqT", [IN_DIM, DVC], bf16, kind="ExternalInput").ap()
    wkT = nc.dram_tensor("wkT", [IN_DIM, DVC], bf16, kind="ExternalInput").ap()
    wvT = nc.dram_tensor("wvT", [IN_DIM, DVC], bf16, kind="ExternalInput").ap()
    woT = nc.dram_tensor("woT", [DVC, DIM], bf16, kind="ExternalInput").ap()
    ones_d = nc.dram_tensor("ones", [P, P], bf16, kind="ExternalInput").ap()
    yT = nc.dram_tensor("yT", [DIM, N], f32, kind="ExternalOutput").ap()

    BW = 512                   # moving-block width (max moving free dim)
    NB = N // BW               # 4 blocks

    with tile.TileContext(nc) as tc:
      for _rep in range(reps):
        with tc.tile_pool(name="persist", bufs=1) as persist:
            qT_sb = persist.tile([P, HH, N], bf16)      # [d%128, head, i]
            kT_sb = persist.tile([P, HH, N], bf16)      # [d%128, head, j]
            v_sb = persist.tile([P, NJT, DVC], bf16)    # [j%128, jt, dv]
            oT_ts = [[persist.tile([P, 1024], bf16, tag=f"o{h}_{bb}",
                                   name=f"o{h}_{bb}")
                      for bb in range(IB2)] for h in range(HH)]
            ones_sb = persist.tile([P, P], bf16)
            wo_sb = persist.tile([P, HH, DIM], bf16)
            nc.sync.dma_start(out=ones_sb, in_=ones_d)

            # ---------------- phase 1: projections ----------------
            with tc.tile_pool(name="wpool", bufs=1) as wpool, \
                 tc.tile_pool(name="xblk", bufs=3) as xblk, \
                 tc.tile_pool(name="psum1", bufs=6, space="PSUM") as psum1:
                wk_sb = wpool.tile([P, KT, DVC], bf16, name="wk")
                wv_sb = wpool.tile([P, KT, DVC], bf16, name="wv")
                wq_sb = wpool.tile([P, KT, DVC], bf16, name="wq")

                for jb in range(NB):
                    xb_blk = xblk.tile([P, KT, BW], bf16, tag="xblk")
                    nc.sync.dma_start(
                        out=xb_blk,
                        in_=xbT[:, jb * BW:(jb + 1) * BW]
                        .rearrange("(kt p) i -> p kt i", p=P))
                    if jb == 0:
                        # weight DMAs queued after the first x block so the
                        # first matmul's operands land earliest
                        nc.sync.dma_start(
                            out=wk_sb,
                            in_=wkT.rearrange("(kt p) d -> p kt d", p=P))
                        nc.sync.dma_start(
                            out=wv_sb,
                            in_=wvT.rearrange("(kt p) d -> p kt d", p=P))
                        nc.sync.dma_start(
                            out=wq_sb,
                            in_=wqT.rearrange("(kt p) d -> p kt d", p=P))
                        # wo prefetch: needed only in phase 3, queued last
                        nc.sync.dma_start(
                            out=wo_sb,
                            in_=woT.rearrange("(dt p) e -> p dt e", p=P))
                    for dt in range(HH):
                        ps = psum1.tile([P, BW], f32, tag="ps1")
                        for kt in range(KT):
                            nc.tensor.matmul(
                                ps,
                                wk_sb[:, kt, dt * P:(dt + 1) * P],
                                xb_blk[:, kt, :],
                                start=(kt == 0), stop=(kt == KT - 1))
                        nc.vector.tensor_copy(
                            kT_sb[:, dt, jb * BW:(jb + 1) * BW], ps)
                    for j2 in range(BW // P):
                        jt = jb * (BW // P) + j2
                        ps = psum1.tile([P, DVC], f32, tag="ps1")
                        for kt in range(KT):
                            nc.tensor.matmul(
                                ps,
                                xb_blk[:, kt, j2 * P:(j2 + 1) * P],
                                wv_sb[:, kt, :],
                                start=(kt == 0), stop=(kt == KT - 1))
                        nc.scalar.copy(v_sb[:, jt, :], ps)

                for ib in range(NB):
                    xa_blk = xblk.tile([P, KT, BW], bf16, tag="xblk")
                    nc.sync.dma_start(
                        out=xa_blk,
                        in_=xaT[:, ib * BW:(ib + 1) * BW]
                        .rearrange("(kt p) i -> p kt i", p=P))
                    for dt in range(HH):
                        ps = psum1.tile([P, BW], f32, tag="ps1")
                        for kt in range(KT):
                            nc.tensor.matmul(
                                ps,
                                wq_sb[:, kt, dt * P:(dt + 1) * P],
                                xa_blk[:, kt, :],
                                start=(kt == 0), stop=(kt == KT - 1))
                        nc.vector.tensor_copy(
                            qT_sb[:, dt, ib * BW:(ib + 1) * BW], ps)

            # ---------------- phase 2: attention ----------------
            with tc.tile_pool(name="expp", bufs=6) as expp, \
                 tc.tile_pool(name="accp", bufs=2) as accp, \
                 tc.tile_pool(name="rcp", bufs=2) as rcp, \
                 tc.tile_pool(name="dotsp", bufs=2, space="PSUM") as dotsp, \
                 tc.tile_pool(name="avp", bufs=1, space="PSUM") as avp, \
                 tc.tile_pool(name="pdp", bufs=1, space="PSUM") as pdp:
                LAG = 2   # PV trails dots/exp by 2 j-tiles so the PE
                          # never waits on the ACT exp of the current tile
                for ib in range(IB2):
                    for h in range(HH):
                        i0 = ib * 1024
                        po = avp.tile([P, 1024], f32)
                        accV = accp.tile([P, 1024], bf16, tag="accV")
                        accG = accp.tile([P, 1024], bf16, tag="accG")
                        ets = {}
                        for jt in range(NJT + LAG):
                            if jt < NJT:
                                ps = dotsp.tile([P, 1024], f32, tag="ps")
                                k_l = kT_sb[:, h, jt * P:(jt + 1) * P]
                                for hf in range(2):
                                    nc.tensor.matmul(
                                        ps[:, hf * 512:(hf + 1) * 512],
                                        k_l,
                                        qT_sb[:, h,
                                              i0 + hf * 512:i0 + (hf + 1) * 512],
                                        start=True, stop=True)
                                et = expp.tile([P, 1024], bf16, tag="exp")
                                nc.scalar.activation(et, ps, Exp, scale=SCALE)
                                ets[jt] = et
                                # softmax denominator partials off the PE:
                                # DVE takes even j-tiles, GpSimd odd ones
                                if jt == 0:
                                    nc.vector.tensor_copy(accV, et)
                                elif jt == 1:
                                    nc.gpsimd.tensor_copy(accG, et)
                                elif jt % 2 == 0:
                                    nc.vector.tensor_add(accV, accV, et)
                                else:
                                    nc.gpsimd.tensor_add(accG, accG, et)
                            if jt >= LAG:
                                jd = jt - LAG
                                et = ets.pop(jd)
                                v_l = v_sb[:, jd, h * P:(h + 1) * P]
                                for hf in range(2):
                                    sl = slice(hf * 512, (hf + 1) * 512)
                                    nc.tensor.matmul(
                                        po[:, sl], v_l, et[:, sl],
                                        start=(jd == 0), stop=(jd == NJT - 1))
                        # partition-reduce the two accumulators with an
                        # all-ones stationary: pd rows all equal the denom
                        pd = pdp.tile([P, 1024], f32)
                        for hf in range(2):
                            sl = slice(hf * 512, (hf + 1) * 512)
                            nc.tensor.matmul(pd[:, sl], ones_sb, accV[:, sl],
                                             start=True, stop=False)
                            nc.tensor.matmul(pd[:, sl], ones_sb, accG[:, sl],
                                             start=False, stop=True)
                        rc = rcp.tile([P, 1024], f32, tag="rc")
                        nc.vector.reciprocal_approx_fast(rc, pd)
                        osl = oT_ts[h][ib]
                        nc.vector.tensor_mul(osl, po, rc)

                # ---------------- phase 3: output projection ----------------
                # y-psum tiles share the dots pool slots (tag "ps"), which
                # free as the exp of the final j-tiles completes
                with tc.tile_pool(name="ystage", bufs=4) as ystage:
                    for ib in range(NIB):
                        bb, half = divmod(ib, 2)
                        for e8 in range(DIM // P):
                            ps = dotsp.tile([P, 512], f32, tag="ps")
                            for dt in range(HH):
                                nc.tensor.matmul(
                                    ps,
                                    wo_sb[:, dt, e8 * P:(e8 + 1) * P],
                                    oT_ts[dt][bb][:, half * 512:(half + 1) * 512],
                                    start=(dt == 0), stop=(dt == HH - 1))
                            ys = ystage.tile([P, 512], f32, tag="ys")
                            if ib % 2 == 0:
                                nc.vector.tensor_copy(ys, ps)
                            else:
                                nc.scalar.copy(ys, ps)
                            nc.sync.dma_start(
                                out=yT[e8 * P:(e8 + 1) * P,
                                       ib * 512:(ib + 1) * 512],
                                in_=ys)

    nc.compile()
    return nc


_nc_by_reps = {}


def _get_nc(reps=1):
    if reps not in _nc_by_reps:
        _nc_by_reps[reps] = _build_nc(reps)
    return _nc_by_reps[reps]


def _make_in_maps(x_a, x_b, W_q, W_kv, W_out):
    import ml_dtypes
    bf16 = ml_dtypes.bfloat16

    xaT = [np.ascontiguousarray(x_a[b].T).astype(bf16) for b in range(B)]
    xbT = [np.ascontiguousarray(x_b[b].T).astype(bf16) for b in range(B)]
    in_maps = []
    for c in range(NCORES):
        b, hh = divmod(c, 2)
        hs = hh * DVC
        in_maps.append({
            "xaT": xaT[b],
            "xbT": xbT[b],
            "wqT": np.ascontiguousarray(W_q[hs:hs + DVC].T).astype(bf16),
            "wkT": np.ascontiguousarray(W_kv[hs:hs + DVC].T).astype(bf16),
            "wvT": np.ascontiguousarray(
                W_kv[DIM + hs:DIM + hs + DVC].T).astype(bf16),
            "woT": np.ascontiguousarray(W_out[:, hs:hs + DVC].T).astype(bf16),
            "ones": np.ones((P, P), dtype=bf16),
        })
    return in_maps


def kernel(x_a, x_b, W_q, W_kv, W_out, b_out):
    global LAST_EXEC_NS, LAST_RESULTS
    from concourse import bass_utils

    x_a = np.asarray(x_a, dtype=np.float32)
    x_b = np.asarray(x_b, dtype=np.float32)
    W_q = np.asarray(W_q, dtype=np.float32)
    W_kv = np.asarray(W_kv, dtype=np.float32)
    W_out = np.asarray(W_out, dtype=np.float32)
    b_out = np.asarray(b_out, dtype=np.float32)

    nc = _get_nc(REPS)
    in_maps = _make_in_maps(x_a, x_b, W_q, W_kv, W_out)

    res = bass_utils.run_bass_kernel_spmd(
        nc, in_maps, core_ids=list(range(NCORES)), trace=_TRACE)
    LAST_EXEC_NS = res.exec_time_ns
    LAST_RESULTS = res

    out = np.empty((B, N, DIM), dtype=np.float32)
    for b in range(B):
        acc = res.results[2 * b]["yT"] + res.results[2 * b + 1]["yT"]
        out[b] = acc.T + b_out
    return out


def bench(inputs, reps_pair=(1, 9), iters=5):
    """Measure on-device time per kernel body via rep-delta wall timing."""
    import time
    from concourse import bass_utils
    ins = {k: np.asarray(v, dtype=np.float32) for k, v in inputs.items()
           if k != "b_out"}
    in_maps = _make_in_maps(ins["x_a"], ins["x_b"], ins["W_q"], ins["W_kv"],
                            ins["W_out"])
    walls = {}
    for reps in reps_pair:
        nc = _get_nc(reps)
        # warm-up (compile+cache)
        bass_utils.run_bass_kernel_spmd(nc, in_maps, core_ids=list(range(NCORES)))
        ts = []
        for _ in range(iters):
            t0 = time.perf_counter()
            bass_utils.run_bass_kernel_spmd(nc, in_maps,
                                            core_ids=list(range(NCORES)))
            ts.append(time.perf_counter() - t0)
        walls[reps] = min(ts)
        print(f"reps={reps}: wall min={walls[reps]*1e3:.2f} ms  all={[f'{t*1e3:.1f}' for t in ts]}")
    r0, r1 = reps_pair
    ns = (walls[r1] - walls[r0]) / (r1 - r0) * 1e9
    print(f"per-body device time: {ns:.0f} ns")
    return ns


# revision 6
# speedup vs baseline: 1.3355x; 1.0757x over previous
"""Cross-modal attention TRN2 kernel (bf16 pipeline).

Problem: B=4, N=2048, IN_DIM=DIM=1024, HEADS=8, D_HEAD=128, scale=DIM**-0.5.
  q = x_a @ W_q.T ; k,v = split(x_b @ W_kv.T) ; per-head softmax(q k^T/32) v ;
  out = merge_heads @ W_out.T + b_out

Sharding over 8 cores: core c -> batch b=c//2, head-half hh=c%2 (4 heads,
512 of DIM).  W_q/W_kv column-sharded, W_out row-sharded (Megatron); each
core emits a partial output projection y_cT = (W_out[:, slice] @ O_half)
of shape [DIM, N]; host sums the two head-half partials per batch, adds
b_out, transposes back.

Device layout: everything transposed ([feature, token]) so all matmuls
contract over the partition dim.  All matmul operands are bf16 (host
casts x and W), PSUM accumulation f32; measured end-to-end rel-err vs the
f32 reference ~5e-3 (tolerance 2e-2).

  phase 1: K^T/V from x_b blocks (shared SBUF block), Q^T from x_a;
           512-wide moving blocks, one [128,512] psum per (dt|jt) with
           8 contraction matmuls.  K/Q psum->SBUF copies on DVE, V on ACT.
  phase 2: per (1024-i-block, head): for each of 16 j-tiles: dots^T
           (2 matmuls), exp on ACT (no max subtraction; |s*scale| < ~1),
           and the softmax denominator accumulated OFF the PE: DVE sums
           even j-tiles into accV, GpSimd odd into accG (elementwise adds
           of the exp tiles).  PV trails by LAG j-tiles in a second psum.
           Epilogue: 4 narrow reduce-matmuls (all-ones stationary) give
           den as a [128,1024] psum (rows identical), DVE
           reciprocal_approx_fast -> rc, one fused DVE multiply
           osl = po * rc drains PV psum.
  phase 3: y^T = WoT.T @ O^T (512-wide), staging copies alternate
           DVE/ACT, DMA -> DRAM f32.
"""

import numpy as np

B, N, IN_DIM, DIM, HEADS = 4, 2048, 1024, 1024, 8
D_HEAD = DIM // HEADS          # 128
SCALE = DIM ** -0.5            # 1/32
NCORES = 8
HH = HEADS // 2                # 4 heads per core
DVC = HH * D_HEAD              # 512 dv per core
P = 128
KT = IN_DIM // P               # 8 contraction tiles
NJT = N // P                   # 16 j tiles
NIB = N // 512                 # 4 i-blocks of 512
IB2 = N // 1024                # 2 i-blocks of 1024

_TRACE = False
REPS = 1
LAST_EXEC_NS = None
LAST_RESULTS = None


def _build_nc(reps=1):
    import concourse.tile as tile
    from concourse import bacc, mybir

    f32 = mybir.dt.float32
    bf16 = mybir.dt.bfloat16
    Exp = mybir.ActivationFunctionType.Exp

    nc = bacc.Bacc("TRN2", debug=False, num_devices=NCORES)

    xaT = nc.dram_tensor("xaT", [IN_DIM, N], bf16, kind="ExternalInput").ap()
    xbT = nc.dram_tensor("xbT", [IN_DIM, N], bf16, kind="ExternalInput").ap()
    wqT = nc.dram_tensor("wqT", [IN_DIM, DVC], bf16, kind="ExternalInput").ap()
    wkT = nc.dram_tensor("wkT", [IN_DIM, DVC], bf16, kind="ExternalInput").ap()
    wvT = nc.dram_tensor("wvT", [IN_DIM, DVC], bf16, kind="ExternalInput").ap()
    woT = nc.dram_tensor("woT", [DVC, DIM], bf16, kind="ExternalInput").ap()
    ones_d = nc.dram_tensor("ones", [P, P], bf16, kind="ExternalInput").ap()
    yT = nc.dram_tensor("yT", [DIM, N], f32, kind="ExternalOutput").ap()

    BW = 512                   # moving-block width (max moving free dim)
    NB = N // BW               # 4 blocks

    with tile.TileContext(nc) as tc:
      for _rep in range(reps):
        with tc.tile_pool(name="persist", bufs=1) as persist:
            qT_sb = persist.tile([P, HH, N], bf16)      # [d%128, head, i]
            kT_sb = persist.tile([P, HH, N], bf16)      # [d%128, head, j]
            v_sb = persist.tile([P, NJT, DVC], bf16)    # [j%128, jt, dv]
            oT_ts = [[persist.tile([P, 1024], bf16, tag=f"o{h}_{bb}",
                                   name=f"o{h}_{bb}")
                      for bb in range(IB2)] for h in range(HH)]
            ones_sb = persist.tile([P, P], bf16)
            wo_sb = persist.tile([P, HH, DIM], bf16)
            nc.sync.dma_start(out=ones_sb, in_=ones_d)

            # ---------------- phase 1: projections ----------------
            with tc.tile_pool(name="wpool", bufs=1) as wpool, \
                 tc.tile_pool(name="xblk", bufs=3) as xblk, \
                 tc.tile_pool(name="psum1", bufs=6, space="PSUM") as psum1:
                wk_sb = wpool.tile([P, KT, DVC], bf16, name="wk")
                wv_sb = wpool.tile([P, KT, DVC], bf16, name="wv")
                wq_sb = wpool.tile([P, KT, DVC], bf16, name="wq")
                # weights ride separate DGE queues so the x blocks on the
                # sync queue aren't delayed behind them
                nc.scalar.dma_start(
                    out=wk_sb, in_=wkT.rearrange("(kt p) d -> p kt d", p=P))
                nc.gpsimd.dma_start(
                    out=wv_sb, in_=wvT.rearrange("(kt p) d -> p kt d", p=P))
                nc.gpsimd.dma_start(
                    out=wq_sb, in_=wqT.rearrange("(kt p) d -> p kt d", p=P))

                for jb in range(NB):
                    xb_blk = xblk.tile([P, KT, BW], bf16, tag="xblk")
                    nc.sync.dma_start(
                        out=xb_blk,
                        in_=xbT[:, jb * BW:(jb + 1) * BW]
                        .rearrange("(kt p) i -> p kt i", p=P))
                    for dt in range(HH):
                        ps = psum1.tile([P, BW], f32, tag="ps1")
                        for kt in range(KT):
                            nc.tensor.matmul(
                                ps,
                                wk_sb[:, kt, dt * P:(dt + 1) * P],
                                xb_blk[:, kt, :],
                                start=(kt == 0), stop=(kt == KT - 1))
                        nc.vector.tensor_copy(
                            kT_sb[:, dt, jb * BW:(jb + 1) * BW], ps)
                    for j2 in range(BW // P):
                        jt = jb * (BW // P) + j2
                        ps = psum1.tile([P, DVC], f32, tag="ps1")
                        for kt in range(KT):
                            nc.tensor.matmul(
                                ps,
                                xb_blk[:, kt, j2 * P:(j2 + 1) * P],
                                wv_sb[:, kt, :],
                                start=(kt == 0), stop=(kt == KT - 1))
                        nc.scalar.copy(v_sb[:, jt, :], ps)

                for ib in range(NB):
                    xa_blk = xblk.tile([P, KT, BW], bf16, tag="xblk")
                    nc.sync.dma_start(
                        out=xa_blk,
                        in_=xaT[:, ib * BW:(ib + 1) * BW]
                        .rearrange("(kt p) i -> p kt i", p=P))
                    if ib == 0:
                        # wo prefetch: needed only in phase 3
                        nc.sync.dma_start(
                            out=wo_sb,
                            in_=woT.rearrange("(dt p) e -> p dt e", p=P))
                    for dt in range(HH):
                        ps = psum1.tile([P, BW], f32, tag="ps1")
                        for kt in range(KT):
                            nc.tensor.matmul(
                                ps,
                                wq_sb[:, kt, dt * P:(dt + 1) * P],
                                xa_blk[:, kt, :],
                                start=(kt == 0), stop=(kt == KT - 1))
                        nc.vector.tensor_copy(
                            qT_sb[:, dt, ib * BW:(ib + 1) * BW], ps)

            # ---------------- phase 2: attention + phase 3 ----------------
            with tc.tile_pool(name="expp", bufs=8) as expp, \
                 tc.tile_pool(name="accp", bufs=2) as accp, \
                 tc.tile_pool(name="rcp", bufs=2) as rcp, \
                 tc.tile_pool(name="ystage", bufs=4) as ystage, \
                 tc.tile_pool(name="dotsp", bufs=2, space="PSUM") as dotsp, \
                 tc.tile_pool(name="avp", bufs=2, space="PSUM") as avp:
                LAG = 2   # PV trails dots/exp by 2 j-tiles so the PE
                          # never waits on the ACT exp of the current tile
                PE_JTS = (13, 14, 15)  # denominator tiles summed by direct
                                       # ones-matmuls at the epilogue; the
                                       # rest go to DVE (even) / GpSimd (odd)

                def emit_epilogue(po, accV, accG, pe_ets, osl):
                    # partition-reduce accumulators + the PE_JTS exp tiles
                    # with an all-ones stationary: pd rows all = the denom.
                    # pd borrows a dots-ring psum slot (released after rc).
                    pd = dotsp.tile([P, 1024], f32, tag="ps")
                    movs = [accV, accG] + pe_ets
                    for hf in range(2):
                        sl = slice(hf * 512, (hf + 1) * 512)
                        for mi, mv in enumerate(movs):
                            nc.tensor.matmul(pd[:, sl], ones_sb, mv[:, sl],
                                             start=(mi == 0),
                                             stop=(mi == len(movs) - 1))
                    rc = rcp.tile([P, 1024], f32, tag="rc")
                    nc.vector.reciprocal_approx_fast(rc, pd)
                    nc.vector.tensor_mul(osl, po, rc)

                def emit_proj_chunk(ibq):
                    # output projection for one 512-wide i-slice; psum from
                    # the dots ring, staging copies split DVE/ACT
                    bb, half = divmod(ibq, 2)
                    for e8 in range(DIM // P):
                        ps = dotsp.tile([P, 512], f32, tag="ps")
                        for dt in range(HH):
                            nc.tensor.matmul(
                                ps,
                                wo_sb[:, dt, e8 * P:(e8 + 1) * P],
                                oT_ts[dt][bb][:, half * 512:(half + 1) * 512],
                                start=(dt == 0), stop=(dt == HH - 1))
                        ys = ystage.tile([P, 512], f32, tag="ys")
                        if e8 % 2 == 0:
                            nc.vector.tensor_copy(ys, ps)
                        else:
                            nc.scalar.copy(ys, ps)
                        nc.sync.dma_start(
                            out=yT[e8 * P:(e8 + 1) * P,
                                   ibq * 512:(ibq + 1) * 512],
                            in_=ys)

                pending = None
                blocks = [(ib, h) for ib in range(IB2) for h in range(HH)]
                for bi, (ib, h) in enumerate(blocks):
                    i0 = ib * 1024
                    po = avp.tile([P, 1024], f32, tag="po")
                    accV = accp.tile([P, 1024], bf16, tag="accV")
                    accG = accp.tile([P, 1024], bf16, tag="accG")
                    ets = {}
                    pe_ets = []
                    for jt in range(NJT + LAG):
                        if jt < NJT:
                            ps = dotsp.tile([P, 1024], f32, tag="ps")
                            k_l = kT_sb[:, h, jt * P:(jt + 1) * P]
                            for hf in range(2):
                                nc.tensor.matmul(
                                    ps[:, hf * 512:(hf + 1) * 512],
                                    k_l,
                                    qT_sb[:, h,
                                          i0 + hf * 512:i0 + (hf + 1) * 512],
                                    start=True, stop=True)
                            et = expp.tile([P, 1024], bf16, tag="exp")
                            nc.scalar.activation(et, ps, Exp, scale=SCALE)
                            ets[jt] = et
                            # softmax denominator partials
                            if jt in PE_JTS:
                                pe_ets.append(et)
                            elif jt == 0:
                                nc.vector.tensor_copy(accV, et)
                            elif jt == 1:
                                nc.vector.tensor_copy(accG, et)
                            elif jt == 12:
                                nc.gpsimd.tensor_add(accG, accG, et)
                            elif jt % 2 == 0:
                                nc.vector.tensor_add(accV, accV, et)
                            else:
                                nc.gpsimd.tensor_add(accG, accG, et)
                        if jt == 3 and pending is not None:
                            # previous block's epilogue, deferred so its
                            # reduce-matmuls never stall the PE pipeline
                            pending()
                            pending = None
                        if jt >= LAG:
                            jd = jt - LAG
                            et = ets.pop(jd)
                            v_l = v_sb[:, jd, h * P:(h + 1) * P]
                            for hf in range(2):
                                sl = slice(hf * 512, (hf + 1) * 512)
                                nc.tensor.matmul(
                                    po[:, sl], v_l, et[:, sl],
                                    start=(jd == 0), stop=(jd == NJT - 1))
                    import functools
                    pending = functools.partial(
                        emit_epilogue, po, accV, accG, pe_ets, oT_ts[h][ib])
                    # first half of the output projection interleaves at the
                    # block boundaries after the ib=0 oT tiles are complete
                    if bi == 4:
                        emit_proj_chunk(0)
                    elif bi == 5:
                        emit_proj_chunk(1)
                pending()
                emit_proj_chunk(2)
                emit_proj_chunk(3)

    nc.compile()
    return nc


_nc_by_reps = {}


def _get_nc(reps=1):
    if reps not in _nc_by_reps:
        _nc_by_reps[reps] = _build_nc(reps)
    return _nc_by_reps[reps]


def _make_in_maps(x_a, x_b, W_q, W_kv, W_out):
    import ml_dtypes
    bf16 = ml_dtypes.bfloat16

    xaT = [np.ascontiguousarray(x_a[b].T).astype(bf16) for b in range(B)]
    xbT = [np.ascontiguousarray(x_b[b].T).astype(bf16) for b in range(B)]
    in_maps = []
    for c in range(NCORES):
        b, hh = divmod(c, 2)
        hs = hh * DVC
        in_maps.append({
            "xaT": xaT[b],
            "xbT": xbT[b],
            "wqT": np.ascontiguousarray(W_q[hs:hs + DVC].T).astype(bf16),
            "wkT": np.ascontiguousarray(W_kv[hs:hs + DVC].T).astype(bf16),
            "wvT": np.ascontiguousarray(
                W_kv[DIM + hs:DIM + hs + DVC].T).astype(bf16),
            "woT": np.ascontiguousarray(W_out[:, hs:hs + DVC].T).astype(bf16),
            "ones": np.ones((P, P), dtype=bf16),
        })
    return in_maps


def kernel(x_a, x_b, W_q, W_kv, W_out, b_out):
    global LAST_EXEC_NS, LAST_RESULTS
    from concourse import bass_utils

    x_a = np.asarray(x_a, dtype=np.float32)
    x_b = np.asarray(x_b, dtype=np.float32)
    W_q = np.asarray(W_q, dtype=np.float32)
    W_kv = np.asarray(W_kv, dtype=np.float32)
    W_out = np.asarray(W_out, dtype=np.float32)
    b_out = np.asarray(b_out, dtype=np.float32)

    nc = _get_nc(REPS)
    in_maps = _make_in_maps(x_a, x_b, W_q, W_kv, W_out)

    res = bass_utils.run_bass_kernel_spmd(
        nc, in_maps, core_ids=list(range(NCORES)), trace=_TRACE)
    LAST_EXEC_NS = res.exec_time_ns
    LAST_RESULTS = res

    out = np.empty((B, N, DIM), dtype=np.float32)
    for b in range(B):
        acc = res.results[2 * b]["yT"] + res.results[2 * b + 1]["yT"]
        out[b] = acc.T + b_out
    return out


def bench(inputs, reps_pair=(1, 9), iters=5):
    """Measure on-device time per kernel body via rep-delta wall timing."""
    import time
    from concourse import bass_utils
    ins = {k: np.asarray(v, dtype=np.float32) for k, v in inputs.items()
           if k != "b_out"}
    in_maps = _make_in_maps(ins["x_a"], ins["x_b"], ins["W_q"], ins["W_kv"],
                            ins["W_out"])
    walls = {}
    for reps in reps_pair:
        nc = _get_nc(reps)
        # warm-up (compile+cache)
        bass_utils.run_bass_kernel_spmd(nc, in_maps, core_ids=list(range(NCORES)))
        ts = []
        for _ in range(iters):
            t0 = time.perf_counter()
            bass_utils.run_bass_kernel_spmd(nc, in_maps,
                                            core_ids=list(range(NCORES)))
            ts.append(time.perf_counter() - t0)
        walls[reps] = min(ts)
        print(f"reps={reps}: wall min={walls[reps]*1e3:.2f} ms  all={[f'{t*1e3:.1f}' for t in ts]}")
    r0, r1 = reps_pair
    ns = (walls[r1] - walls[r0]) / (r1 - r0) * 1e9
    print(f"per-body device time: {ns:.0f} ns")
    return ns


# revision 16
# speedup vs baseline: 1.3943x; 1.0440x over previous
"""Cross-modal attention TRN2 kernel (fp16 pipeline).

Problem: B=4, N=2048, IN_DIM=DIM=1024, HEADS=8, D_HEAD=128, scale=DIM**-0.5.
  q = x_a @ W_q.T ; k,v = split(x_b @ W_kv.T) ; per-head softmax(q k^T/32) v ;
  out = merge_heads @ W_out.T + b_out

Sharding over 8 cores: core c -> batch b=c//2, head-half hh=c%2 (4 heads,
512 of DIM).  W_q/W_kv column-sharded, W_out row-sharded (Megatron); each
core emits a partial output projection y_cT = (W_out[:, slice] @ O_half)
of shape [DIM, N]; host sums the two head-half partials per batch, adds
b_out, transposes back.

Device layout: everything transposed ([feature, token]) so all matmuls
contract over the partition dim.  All matmul operands are fp16 (host
casts x and W), PSUM accumulation f32, y written back fp16; simulated
end-to-end rel-err vs the f32 reference ~2.4e-3 (tolerance 2e-2).

  phase 1: K^T/V from x_b blocks (shared SBUF block), Q^T from x_a;
           512-wide moving blocks, one [128,512] psum per (dt|jt) with
           8 contraction matmuls.  K/Q psum->SBUF copies on DVE, V on ACT.
  phase 2: per (1024-i-block, head): for each of 16 j-tiles: dots^T
           (2 matmuls), exp on ACT (no max subtraction; |s*scale| < ~1),
           and the softmax denominator accumulated OFF the PE: DVE sums
           even j-tiles into accV, GpSimd odd into accG (elementwise adds
           of the exp tiles).  PV trails by LAG j-tiles in a second psum.
           Epilogue: 4 narrow reduce-matmuls (all-ones stationary) give
           den as a [128,1024] psum (rows identical), DVE
           reciprocal_approx_fast -> rc, one fused DVE multiply
           osl = po * rc drains PV psum.
  phase 3: y^T = WoT.T @ O^T (512-wide), staging copies alternate
           DVE/ACT, DMA -> DRAM f32.
"""

import numpy as np

B, N, IN_DIM, DIM, HEADS = 4, 2048, 1024, 1024, 8
D_HEAD = DIM // HEADS          # 128
SCALE = DIM ** -0.5            # 1/32
NCORES = 8
HH = HEADS // 2                # 4 heads per core
DVC = HH * D_HEAD              # 512 dv per core
P = 128
KT = IN_DIM // P               # 8 contraction tiles
NJT = N // P                   # 16 j tiles
NIB = N // 512                 # 4 i-blocks of 512
IB2 = N // 1024                # 2 i-blocks of 1024

_TRACE = False
REPS = 1
LAST_EXEC_NS = None
LAST_RESULTS = None


def _build_nc(reps=1):
    import concourse.tile as tile
    from concourse import bacc, mybir

    f32 = mybir.dt.float32
    f16 = mybir.dt.float16
    Exp = mybir.ActivationFunctionType.Exp

    nc = bacc.Bacc("TRN2", debug=False, num_devices=NCORES)

    xaT = nc.dram_tensor("xaT", [IN_DIM, N], f16, kind="ExternalInput").ap()
    xbT = nc.dram_tensor("xbT", [IN_DIM, N], f16, kind="ExternalInput").ap()
    wqT = nc.dram_tensor("wqT", [IN_DIM, DVC], f16, kind="ExternalInput").ap()
    wkT = nc.dram_tensor("wkT", [IN_DIM, DVC], f16, kind="ExternalInput").ap()
    wvT = nc.dram_tensor("wvT", [IN_DIM, DVC], f16, kind="ExternalInput").ap()
    woT = nc.dram_tensor("woT", [DVC, DIM], f16, kind="ExternalInput").ap()
    ones_d = nc.dram_tensor("ones", [P, P], f16, kind="ExternalInput").ap()
    yT = nc.dram_tensor("yT", [DIM, N], f16, kind="ExternalOutput").ap()

    BW = 512                   # moving-block width (max moving free dim)
    NB = N // BW               # 4 blocks

    with tile.TileContext(nc) as tc:
      for _rep in range(reps):
        with tc.tile_pool(name="persist", bufs=1) as persist:
            qT_sb = persist.tile([P, HH, N], f16)      # [d%128, head, i]
            kT_sb = persist.tile([P, HH, N], f16)      # [d%128, head, j]
            v_sb = persist.tile([P, NJT, DVC], f16)    # [j%128, jt, dv]
            oT_ts = [[persist.tile([P, 1024], f16, tag=f"o{h}_{bb}",
                                   name=f"o{h}_{bb}")
                      for bb in range(IB2)] for h in range(HH)]
            ones_sb = persist.tile([P, P], f16)
            wo_sb = persist.tile([P, HH, DIM], f16)
            nc.sync.dma_start(out=ones_sb, in_=ones_d)

            # ---------------- phase 1: projections ----------------
            with tc.tile_pool(name="wpool", bufs=1) as wpool, \
                 tc.tile_pool(name="xblk", bufs=3) as xblk, \
                 tc.tile_pool(name="psum1", bufs=6, space="PSUM") as psum1:
                wk_sb = wpool.tile([P, KT, DVC], f16, name="wk")
                wv_sb = wpool.tile([P, KT, DVC], f16, name="wv")
                wq_sb = wpool.tile([P, KT, DVC], f16, name="wq")
                # single in-order sync DGE queue, sequenced by first use:
                # the dt=0 slice of wk (256KB) unblocks the very first
                # matmul, then the first x block, then the rest
                nc.sync.dma_start(
                    out=wk_sb[:, :, 0:P],
                    in_=wkT[:, 0:P].rearrange("(kt p) d -> p kt d", p=P))

                for jb in range(NB):
                    xb_blk = xblk.tile([P, KT, BW], f16, tag="xblk")
                    nc.sync.dma_start(
                        out=xb_blk,
                        in_=xbT[:, jb * BW:(jb + 1) * BW]
                        .rearrange("(kt p) i -> p kt i", p=P))
                    if jb == 0:
                        nc.sync.dma_start(
                            out=wk_sb[:, :, P:DVC],
                            in_=wkT[:, P:DVC]
                            .rearrange("(kt p) d -> p kt d", p=P))
                        nc.sync.dma_start(
                            out=wv_sb,
                            in_=wvT.rearrange("(kt p) d -> p kt d", p=P))
                    elif jb == 1:
                        nc.sync.dma_start(
                            out=wq_sb,
                            in_=wqT.rearrange("(kt p) d -> p kt d", p=P))
                    for dt in range(HH):
                        ps = psum1.tile([P, BW], f32, tag="ps1")
                        for kt in range(KT):
                            nc.tensor.matmul(
                                ps,
                                wk_sb[:, kt, dt * P:(dt + 1) * P],
                                xb_blk[:, kt, :],
                                start=(kt == 0), stop=(kt == KT - 1))
                        nc.vector.tensor_copy(
                            kT_sb[:, dt, jb * BW:(jb + 1) * BW], ps)
                    for j2 in range(BW // P):
                        jt = jb * (BW // P) + j2
                        ps = psum1.tile([P, DVC], f32, tag="ps1")
                        for kt in range(KT):
                            nc.tensor.matmul(
                                ps,
                                xb_blk[:, kt, j2 * P:(j2 + 1) * P],
                                wv_sb[:, kt, :],
                                start=(kt == 0), stop=(kt == KT - 1))
                        nc.scalar.copy(v_sb[:, jt, :], ps)

                for ib in range(NB):
                    xa_blk = xblk.tile([P, KT, BW], f16, tag="xblk")
                    nc.sync.dma_start(
                        out=xa_blk,
                        in_=xaT[:, ib * BW:(ib + 1) * BW]
                        .rearrange("(kt p) i -> p kt i", p=P))
                    if ib == 0:
                        # wo prefetch: needed only in phase 3
                        nc.sync.dma_start(
                            out=wo_sb,
                            in_=woT.rearrange("(dt p) e -> p dt e", p=P))
                    for dt in range(HH):
                        ps = psum1.tile([P, BW], f32, tag="ps1")
                        for kt in range(KT):
                            nc.tensor.matmul(
                                ps,
                                wq_sb[:, kt, dt * P:(dt + 1) * P],
                                xa_blk[:, kt, :],
                                start=(kt == 0), stop=(kt == KT - 1))
                        nc.vector.tensor_copy(
                            qT_sb[:, dt, ib * BW:(ib + 1) * BW], ps)

            # ---------------- phase 2: attention + phase 3 ----------------
            with tc.tile_pool(name="expp", bufs=8) as expp, \
                 tc.tile_pool(name="accp", bufs=2) as accp, \
                 tc.tile_pool(name="rcp", bufs=2) as rcp, \
                 tc.tile_pool(name="ystage", bufs=4) as ystage, \
                 tc.tile_pool(name="dotsp", bufs=2, space="PSUM") as dotsp, \
                 tc.tile_pool(name="avp", bufs=2, space="PSUM") as avp:
                LAG = 2   # PV trails dots/exp by 2 j-tiles so the PE
                          # never waits on the ACT exp of the current tile
                PE_JTS = (14, 15)      # denominator tiles summed by direct
                                       # ones-matmuls at the epilogue; the
                                       # rest go to DVE (even) / GpSimd (odd)

                def emit_epilogue(po, accV, accG, pe_ets, osl):
                    # partition-reduce accumulators + the PE_JTS exp tiles
                    # with an all-ones stationary: pd rows all = the denom.
                    # pd borrows a dots-ring psum slot (released after rc).
                    pd = dotsp.tile([P, 1024], f32, tag="ps")
                    movs = [accV, accG] + pe_ets
                    for hf in range(2):
                        sl = slice(hf * 512, (hf + 1) * 512)
                        for mi, mv in enumerate(movs):
                            nc.tensor.matmul(pd[:, sl], ones_sb, mv[:, sl],
                                             start=(mi == 0),
                                             stop=(mi == len(movs) - 1))
                    rc = rcp.tile([P, 1024], f32, tag="rc")
                    nc.vector.reciprocal_approx_fast(rc, pd)
                    nc.vector.tensor_mul(osl, po, rc)

                def emit_proj_chunk(ibq):
                    # output projection for one 512-wide i-slice; psum from
                    # the dots ring, staging copies split DVE/ACT
                    bb, half = divmod(ibq, 2)
                    for e8 in range(DIM // P):
                        ps = dotsp.tile([P, 512], f32, tag="ps")
                        for dt in range(HH):
                            nc.tensor.matmul(
                                ps,
                                wo_sb[:, dt, e8 * P:(e8 + 1) * P],
                                oT_ts[dt][bb][:, half * 512:(half + 1) * 512],
                                start=(dt == 0), stop=(dt == HH - 1))
                        ys = ystage.tile([P, 512], f16, tag="ys")
                        if e8 % 2 == 0:
                            nc.vector.tensor_copy(ys, ps)
                        else:
                            nc.scalar.copy(ys, ps)
                        nc.sync.dma_start(
                            out=yT[e8 * P:(e8 + 1) * P,
                                   ibq * 512:(ibq + 1) * 512],
                            in_=ys)

                pending = None
                blocks = [(ib, h) for ib in range(IB2) for h in range(HH)]
                for bi, (ib, h) in enumerate(blocks):
                    i0 = ib * 1024
                    po = avp.tile([P, 1024], f32, tag="po")
                    accV = accp.tile([P, 1024], f16, tag="accV")
                    accG = accp.tile([P, 1024], f16, tag="accG")
                    ets = {}
                    pe_ets = []
                    for jt in range(NJT + LAG):
                        if jt < NJT:
                            ps = dotsp.tile([P, 1024], f32, tag="ps")
                            k_l = kT_sb[:, h, jt * P:(jt + 1) * P]
                            for hf in range(2):
                                nc.tensor.matmul(
                                    ps[:, hf * 512:(hf + 1) * 512],
                                    k_l,
                                    qT_sb[:, h,
                                          i0 + hf * 512:i0 + (hf + 1) * 512],
                                    start=True, stop=True)
                            et = expp.tile([P, 1024], f16, tag="exp")
                            nc.scalar.activation(et, ps, Exp, scale=SCALE)
                            ets[jt] = et
                            # softmax denominator partials
                            if jt in PE_JTS:
                                pe_ets.append(et)
                            elif jt == 0:
                                nc.vector.tensor_copy(accV, et)
                            elif jt == 1:
                                nc.vector.tensor_copy(accG, et)
                            elif (jt % 2 == 0) != (jt in (12, 13)):
                                # DVE: evens 2-10 plus 13; GpSimd: odds 3-11
                                # plus 12 (6 adds each)
                                nc.vector.tensor_add(accV, accV, et)
                            else:
                                nc.gpsimd.tensor_add(accG, accG, et)
                        if jt == 3 and pending is not None:
                            # previous block's epilogue, deferred so its
                            # reduce-matmuls never stall the PE pipeline
                            pending()
                            pending = None
                        if jt >= LAG:
                            jd = jt - LAG
                            et = ets.pop(jd)
                            v_l = v_sb[:, jd, h * P:(h + 1) * P]
                            for hf in range(2):
                                sl = slice(hf * 512, (hf + 1) * 512)
                                nc.tensor.matmul(
                                    po[:, sl], v_l, et[:, sl],
                                    start=(jd == 0), stop=(jd == NJT - 1))
                    import functools
                    pending = functools.partial(
                        emit_epilogue, po, accV, accG, pe_ets, oT_ts[h][ib])
                    # first half of the output projection interleaves at the
                    # block boundaries after the ib=0 oT tiles are complete
                    if bi == 4:
                        emit_proj_chunk(0)
                    elif bi == 5:
                        emit_proj_chunk(1)
                pending()
                emit_proj_chunk(2)
                emit_proj_chunk(3)

    nc.compile()
    return nc


_nc_by_reps = {}


def _get_nc(reps=1):
    if reps not in _nc_by_reps:
        _nc_by_reps[reps] = _build_nc(reps)
    return _nc_by_reps[reps]


def _make_in_maps(x_a, x_b, W_q, W_kv, W_out):
    f16 = np.float16

    xaT = [np.ascontiguousarray(x_a[b].T).astype(f16) for b in range(B)]
    xbT = [np.ascontiguousarray(x_b[b].T).astype(f16) for b in range(B)]
    in_maps = []
    for c in range(NCORES):
        b, hh = divmod(c, 2)
        hs = hh * DVC
        in_maps.append({
            "xaT": xaT[b],
            "xbT": xbT[b],
            "wqT": np.ascontiguousarray(W_q[hs:hs + DVC].T).astype(f16),
            "wkT": np.ascontiguousarray(W_kv[hs:hs + DVC].T).astype(f16),
            "wvT": np.ascontiguousarray(
                W_kv[DIM + hs:DIM + hs + DVC].T).astype(f16),
            "woT": np.ascontiguousarray(W_out[:, hs:hs + DVC].T).astype(f16),
            "ones": np.ones((P, P), dtype=f16),
        })
    return in_maps


def kernel(x_a, x_b, W_q, W_kv, W_out, b_out):
    global LAST_EXEC_NS, LAST_RESULTS
    from concourse import bass_utils

    x_a = np.asarray(x_a, dtype=np.float32)
    x_b = np.asarray(x_b, dtype=np.float32)
    W_q = np.asarray(W_q, dtype=np.float32)
    W_kv = np.asarray(W_kv, dtype=np.float32)
    W_out = np.asarray(W_out, dtype=np.float32)
    b_out = np.asarray(b_out, dtype=np.float32)

    nc = _get_nc(REPS)
    in_maps = _make_in_maps(x_a, x_b, W_q, W_kv, W_out)

    res = bass_utils.run_bass_kernel_spmd(
        nc, in_maps, core_ids=list(range(NCORES)), trace=_TRACE)
    LAST_EXEC_NS = res.exec_time_ns
    LAST_RESULTS = res

    out = np.empty((B, N, DIM), dtype=np.float32)
    for b in range(B):
        acc = (res.results[2 * b]["yT"].astype(np.float32)
               + res.results[2 * b + 1]["yT"].astype(np.float32))
        out[b] = acc.T + b_out
    return out


def bench(inputs, reps_pair=(1, 9), iters=5):
    """Measure on-device time per kernel body via rep-delta wall timing."""
    import time
    from concourse import bass_utils
    ins = {k: np.asarray(v, dtype=np.float32) for k, v in inputs.items()
           if k != "b_out"}
    in_maps = _make_in_maps(ins["x_a"], ins["x_b"], ins["W_q"], ins["W_kv"],
                            ins["W_out"])
    walls = {}
    for reps in reps_pair:
        nc = _get_nc(reps)
        # warm-up (compile+cache)
        bass_utils.run_bass_kernel_spmd(nc, in_maps, core_ids=list(range(NCORES)))
        ts = []
        for _ in range(iters):
            t0 = time.perf_counter()
            bass_utils.run_bass_kernel_spmd(nc, in_maps,
                                            core_ids=list(range(NCORES)))
            ts.append(time.perf_counter() - t0)
        walls[reps] = min(ts)
        print(f"reps={reps}: wall min={walls[reps]*1e3:.2f} ms  all={[f'{t*1e3:.1f}' for t in ts]}")
    r0, r1 = reps_pair
    ns = (walls[r1] - walls[r0]) / (r1 - r0) * 1e9
    print(f"per-body device time: {ns:.0f} ns")
    return ns


# revision 17
# speedup vs baseline: 1.4514x; 1.0409x over previous
"""Cross-modal attention TRN2 kernel (fp16 pipeline).

Problem: B=4, N=2048, IN_DIM=DIM=1024, HEADS=8, D_HEAD=128, scale=DIM**-0.5.
  q = x_a @ W_q.T ; k,v = split(x_b @ W_kv.T) ; per-head softmax(q k^T/32) v ;
  out = merge_heads @ W_out.T + b_out

Sharding over 8 cores: core c -> batch b=c//2, head-half hh=c%2 (4 heads,
512 of DIM).  W_q/W_kv column-sharded, W_out row-sharded (Megatron); each
core emits a partial output projection y_cT = (W_out[:, slice] @ O_half)
of shape [DIM, N]; host sums the two head-half partials per batch, adds
b_out, transposes back.

Device layout: everything transposed ([feature, token]) so all matmuls
contract over the partition dim.  All matmul operands are fp16 (host
casts x and W), PSUM accumulation f32, y written back fp16; simulated
end-to-end rel-err vs the f32 reference ~2.4e-3 (tolerance 2e-2).

  phase 1: K^T/V from x_b blocks (shared SBUF block), Q^T from x_a;
           512-wide moving blocks, one [128,512] psum per (dt|jt) with
           8 contraction matmuls.  K/Q psum->SBUF copies on DVE, V on ACT.
  phase 2: per (1024-i-block, head): for each of 16 j-tiles: dots^T
           (2 matmuls), exp on ACT (no max subtraction; |s*scale| < ~1),
           and the softmax denominator accumulated OFF the PE: DVE sums
           even j-tiles into accV, GpSimd odd into accG (elementwise adds
           of the exp tiles).  PV trails by LAG j-tiles in a second psum.
           Epilogue: 4 narrow reduce-matmuls (all-ones stationary) give
           den as a [128,1024] psum (rows identical), DVE
           reciprocal_approx_fast -> rc, one fused DVE multiply
           osl = po * rc drains PV psum.
  phase 3: y^T = WoT.T @ O^T (512-wide), staging copies alternate
           DVE/ACT, DMA -> DRAM f32.
"""

import numpy as np

B, N, IN_DIM, DIM, HEADS = 4, 2048, 1024, 1024, 8
D_HEAD = DIM // HEADS          # 128
SCALE = DIM ** -0.5            # 1/32
NCORES = 8
HH = HEADS // 2                # 4 heads per core
DVC = HH * D_HEAD              # 512 dv per core
P = 128
KT = IN_DIM // P               # 8 contraction tiles
NJT = N // P                   # 16 j tiles
NIB = N // 512                 # 4 i-blocks of 512
IB2 = N // 1024                # 2 i-blocks of 1024

_TRACE = False
REPS = 1
LAST_EXEC_NS = None
LAST_RESULTS = None


def _build_nc(reps=1):
    import concourse.tile as tile
    from concourse import bacc, mybir

    f32 = mybir.dt.float32
    f16 = mybir.dt.float16
    f8 = mybir.dt.float8e4
    DR = mybir.MatmulPerfMode.DoubleRow
    Exp = mybir.ActivationFunctionType.Exp

    nc = bacc.Bacc("TRN2", debug=False, num_devices=NCORES)

    xaT = nc.dram_tensor("xaT", [IN_DIM, N], f8, kind="ExternalInput").ap()
    xbT = nc.dram_tensor("xbT", [IN_DIM, N], f16, kind="ExternalInput").ap()
    wqT = nc.dram_tensor("wqT", [IN_DIM, DVC], f8, kind="ExternalInput").ap()
    wkT = nc.dram_tensor("wkT", [IN_DIM, DVC], f16, kind="ExternalInput").ap()
    wvT = nc.dram_tensor("wvT", [IN_DIM, DVC], f16, kind="ExternalInput").ap()
    woT = nc.dram_tensor("woT", [DVC, DIM], f16, kind="ExternalInput").ap()
    ones_d = nc.dram_tensor("ones", [P, P], f16, kind="ExternalInput").ap()
    yT = nc.dram_tensor("yT", [DIM, N], f16, kind="ExternalOutput").ap()

    BW = 512                   # moving-block width (max moving free dim)
    NB = N // BW               # 4 blocks

    with tile.TileContext(nc) as tc:
      for _rep in range(reps):
        with tc.tile_pool(name="persist", bufs=1) as persist:
            qT_sb = persist.tile([P, HH, N], f16)      # [d%128, head, i]
            kT_sb = persist.tile([P, HH, N], f16)      # [d%128, head, j]
            v_sb = persist.tile([P, NJT, DVC], f16)    # [j%128, jt, dv]
            oT_ts = [[persist.tile([P, 1024], f16, tag=f"o{h}_{bb}",
                                   name=f"o{h}_{bb}")
                      for bb in range(IB2)] for h in range(HH)]
            ones_sb = persist.tile([P, P], f16)
            wo_sb = persist.tile([P, HH, DIM], f16)
            nc.sync.dma_start(out=ones_sb, in_=ones_d)

            # ---------------- phase 1: projections ----------------
            with tc.tile_pool(name="wpool", bufs=1) as wpool, \
                 tc.tile_pool(name="xblk", bufs=3) as xblk, \
                 tc.tile_pool(name="psum1", bufs=6, space="PSUM") as psum1:
                wk_sb = wpool.tile([P, KT, DVC], f16, name="wk")
                wv_sb = wpool.tile([P, KT, DVC], f16, name="wv")
                wq_sb = wpool.tile([P, KT, DVC], f8, name="wq")
                # single in-order sync DGE queue, sequenced by first use:
                # the dt=0 slice of wk (256KB) unblocks the very first
                # matmul, then the first x block, then the rest
                nc.sync.dma_start(
                    out=wk_sb[:, :, 0:P],
                    in_=wkT[:, 0:P].rearrange("(kt p) d -> p kt d", p=P))

                for jb in range(NB):
                    xb_blk = xblk.tile([P, KT, BW], f16, tag="xblk")
                    nc.sync.dma_start(
                        out=xb_blk,
                        in_=xbT[:, jb * BW:(jb + 1) * BW]
                        .rearrange("(kt p) i -> p kt i", p=P))
                    if jb == 0:
                        nc.sync.dma_start(
                            out=wk_sb[:, :, P:DVC],
                            in_=wkT[:, P:DVC]
                            .rearrange("(kt p) d -> p kt d", p=P))
                        nc.sync.dma_start(
                            out=wv_sb,
                            in_=wvT.rearrange("(kt p) d -> p kt d", p=P))
                    elif jb == 1:
                        nc.sync.dma_start(
                            out=wq_sb,
                            in_=wqT.rearrange("(kt p) d -> p kt d", p=P))
                    for dt in range(HH):
                        ps = psum1.tile([P, BW], f32, tag="ps1")
                        for kt in range(KT):
                            nc.tensor.matmul(
                                ps,
                                wk_sb[:, kt, dt * P:(dt + 1) * P],
                                xb_blk[:, kt, :],
                                start=(kt == 0), stop=(kt == KT - 1))
                        nc.vector.tensor_copy(
                            kT_sb[:, dt, jb * BW:(jb + 1) * BW], ps)
                    for j2 in range(BW // P):
                        jt = jb * (BW // P) + j2
                        ps = psum1.tile([P, DVC], f32, tag="ps1")
                        for kt in range(KT):
                            nc.tensor.matmul(
                                ps,
                                xb_blk[:, kt, j2 * P:(j2 + 1) * P],
                                wv_sb[:, kt, :],
                                start=(kt == 0), stop=(kt == KT - 1))
                        nc.scalar.copy(v_sb[:, jt, :], ps)

                for ib in range(NB):
                    xa_blk = xblk.tile([P, KT, BW], f8, tag="xa8", name="xa8")
                    nc.sync.dma_start(
                        out=xa_blk,
                        in_=xaT[:, ib * BW:(ib + 1) * BW]
                        .rearrange("(kt p) i -> p kt i", p=P))
                    if ib == 0:
                        # wo prefetch: needed only in phase 3
                        nc.sync.dma_start(
                            out=wo_sb,
                            in_=woT.rearrange("(dt p) e -> p dt e", p=P))
                    for dt in range(HH):
                        ps = psum1.tile([P, BW], f32, tag="ps1")
                        for kp in range(KT // 2):
                            # fp8 DoubleRow: two 128-deep k-tiles per pass,
                            # 2x PE rate; W_q is host-scaled by 16 so the
                            # psum holds 16*q (folded into the exp scale)
                            nc.tensor.matmul(
                                ps,
                                wq_sb[:, 2 * kp:2 * kp + 2, dt * P:(dt + 1) * P],
                                xa_blk[:, 2 * kp:2 * kp + 2, :],
                                start=(kp == 0), stop=(kp == KT // 2 - 1),
                                perf_mode=DR)
                        nc.vector.tensor_copy(
                            qT_sb[:, dt, ib * BW:(ib + 1) * BW], ps)

            # ---------------- phase 2: attention + phase 3 ----------------
            with tc.tile_pool(name="expp", bufs=8) as expp, \
                 tc.tile_pool(name="accp", bufs=2) as accp, \
                 tc.tile_pool(name="rcp", bufs=2) as rcp, \
                 tc.tile_pool(name="ystage", bufs=4) as ystage, \
                 tc.tile_pool(name="dotsp", bufs=2, space="PSUM") as dotsp, \
                 tc.tile_pool(name="avp", bufs=2, space="PSUM") as avp:
                LAG = 2   # PV trails dots/exp by 2 j-tiles so the PE
                          # never waits on the ACT exp of the current tile
                PE_JTS = (14, 15)      # denominator tiles summed by direct
                                       # ones-matmuls at the epilogue; the
                                       # rest go to DVE (even) / GpSimd (odd)

                def emit_epilogue(po, accV, accG, pe_ets, osl):
                    # partition-reduce accumulators + the PE_JTS exp tiles
                    # with an all-ones stationary: pd rows all = the denom.
                    # pd borrows a dots-ring psum slot (released after rc).
                    pd = dotsp.tile([P, 1024], f32, tag="ps")
                    movs = [accV, accG] + pe_ets
                    for hf in range(2):
                        sl = slice(hf * 512, (hf + 1) * 512)
                        for mi, mv in enumerate(movs):
                            nc.tensor.matmul(pd[:, sl], ones_sb, mv[:, sl],
                                             start=(mi == 0),
                                             stop=(mi == len(movs) - 1))
                    rc = rcp.tile([P, 1024], f32, tag="rc")
                    nc.vector.reciprocal_approx_fast(rc, pd)
                    nc.vector.tensor_mul(osl, po, rc)

                def emit_proj_chunk(ibq):
                    # output projection for one 512-wide i-slice; psum from
                    # the dots ring, staging copies split DVE/ACT
                    bb, half = divmod(ibq, 2)
                    for e8 in range(DIM // P):
                        ps = dotsp.tile([P, 512], f32, tag="ps")
                        for dt in range(HH):
                            nc.tensor.matmul(
                                ps,
                                wo_sb[:, dt, e8 * P:(e8 + 1) * P],
                                oT_ts[dt][bb][:, half * 512:(half + 1) * 512],
                                start=(dt == 0), stop=(dt == HH - 1))
                        ys = ystage.tile([P, 512], f16, tag="ys")
                        if e8 % 2 == 0:
                            nc.vector.tensor_copy(ys, ps)
                        else:
                            nc.scalar.copy(ys, ps)
                        nc.sync.dma_start(
                            out=yT[e8 * P:(e8 + 1) * P,
                                   ibq * 512:(ibq + 1) * 512],
                            in_=ys)

                pending = None
                blocks = [(ib, h) for ib in range(IB2) for h in range(HH)]
                for bi, (ib, h) in enumerate(blocks):
                    i0 = ib * 1024
                    po = avp.tile([P, 1024], f32, tag="po")
                    accV = accp.tile([P, 1024], f16, tag="accV")
                    accG = accp.tile([P, 1024], f16, tag="accG")
                    ets = {}
                    pe_ets = []
                    for jt in range(NJT + LAG):
                        if jt < NJT:
                            ps = dotsp.tile([P, 1024], f32, tag="ps")
                            k_l = kT_sb[:, h, jt * P:(jt + 1) * P]
                            for hf in range(2):
                                nc.tensor.matmul(
                                    ps[:, hf * 512:(hf + 1) * 512],
                                    k_l,
                                    qT_sb[:, h,
                                          i0 + hf * 512:i0 + (hf + 1) * 512],
                                    start=True, stop=True)
                            et = expp.tile([P, 1024], f16, tag="exp")
                            nc.scalar.activation(et, ps, Exp, scale=SCALE / 16.0)
                            ets[jt] = et
                            # softmax denominator partials
                            if jt in PE_JTS:
                                pe_ets.append(et)
                            elif jt == 0:
                                nc.vector.tensor_copy(accV, et)
                            elif jt == 1:
                                nc.vector.tensor_copy(accG, et)
                            elif (jt % 2 == 0) != (jt in (12, 13)):
                                # DVE: evens 2-10 plus 13; GpSimd: odds 3-11
                                # plus 12 (6 adds each)
                                nc.vector.tensor_add(accV, accV, et)
                            else:
                                nc.gpsimd.tensor_add(accG, accG, et)
                        if jt == 3 and pending is not None:
                            # previous block's epilogue, deferred so its
                            # reduce-matmuls never stall the PE pipeline
                            pending()
                            pending = None
                        if jt >= LAG:
                            jd = jt - LAG
                            et = ets.pop(jd)
                            v_l = v_sb[:, jd, h * P:(h + 1) * P]
                            for hf in range(2):
                                sl = slice(hf * 512, (hf + 1) * 512)
                                nc.tensor.matmul(
                                    po[:, sl], v_l, et[:, sl],
                                    start=(jd == 0), stop=(jd == NJT - 1))
                    import functools
                    pending = functools.partial(
                        emit_epilogue, po, accV, accG, pe_ets, oT_ts[h][ib])
                    # first half of the output projection interleaves at the
                    # block boundaries after the ib=0 oT tiles are complete
                    if bi == 4:
                        emit_proj_chunk(0)
                    elif bi == 5:
                        emit_proj_chunk(1)
                pending()
                emit_proj_chunk(2)
                emit_proj_chunk(3)

    nc.compile()
    return nc


_nc_by_reps = {}


def _get_nc(reps=1):
    if reps not in _nc_by_reps:
        _nc_by_reps[reps] = _build_nc(reps)
    return _nc_by_reps[reps]


def _make_in_maps(x_a, x_b, W_q, W_kv, W_out):
    import ml_dtypes
    f16 = np.float16
    f8 = ml_dtypes.float8_e4m3

    xaT = [np.ascontiguousarray(x_a[b].T).astype(f8) for b in range(B)]
    xbT = [np.ascontiguousarray(x_b[b].T).astype(f16) for b in range(B)]
    in_maps = []
    for c in range(NCORES):
        b, hh = divmod(c, 2)
        hs = hh * DVC
        in_maps.append({
            "xaT": xaT[b],
            "xbT": xbT[b],
            "wqT": np.ascontiguousarray(16.0 * W_q[hs:hs + DVC].T).astype(f8),
            "wkT": np.ascontiguousarray(W_kv[hs:hs + DVC].T).astype(f16),
            "wvT": np.ascontiguousarray(
                W_kv[DIM + hs:DIM + hs + DVC].T).astype(f16),
            "woT": np.ascontiguousarray(W_out[:, hs:hs + DVC].T).astype(f16),
            "ones": np.ones((P, P), dtype=f16),
        })
    return in_maps


def kernel(x_a, x_b, W_q, W_kv, W_out, b_out):
    global LAST_EXEC_NS, LAST_RESULTS
    from concourse import bass_utils

    x_a = np.asarray(x_a, dtype=np.float32)
    x_b = np.asarray(x_b, dtype=np.float32)
    W_q = np.asarray(W_q, dtype=np.float32)
    W_kv = np.asarray(W_kv, dtype=np.float32)
    W_out = np.asarray(W_out, dtype=np.float32)
    b_out = np.asarray(b_out, dtype=np.float32)

    nc = _get_nc(REPS)
    in_maps = _make_in_maps(x_a, x_b, W_q, W_kv, W_out)

    res = bass_utils.run_bass_kernel_spmd(
        nc, in_maps, core_ids=list(range(NCORES)), trace=_TRACE)
    LAST_EXEC_NS = res.exec_time_ns
    LAST_RESULTS = res

    out = np.empty((B, N, DIM), dtype=np.float32)
    for b in range(B):
        acc = (res.results[2 * b]["yT"].astype(np.float32)
               + res.results[2 * b + 1]["yT"].astype(np.float32))
        out[b] = acc.T + b_out
    return out


def bench(inputs, reps_pair=(1, 9), iters=5):
    """Measure on-device time per kernel body via rep-delta wall timing."""
    import time
    from concourse import bass_utils
    ins = {k: np.asarray(v, dtype=np.float32) for k, v in inputs.items()
           if k != "b_out"}
    in_maps = _make_in_maps(ins["x_a"], ins["x_b"], ins["W_q"], ins["W_kv"],
                            ins["W_out"])
    walls = {}
    for reps in reps_pair:
        nc = _get_nc(reps)
        # warm-up (compile+cache)
        bass_utils.run_bass_kernel_spmd(nc, in_maps, core_ids=list(range(NCORES)))
        ts = []
        for _ in range(iters):
            t0 = time.perf_counter()
            bass_utils.run_bass_kernel_spmd(nc, in_maps,
                                            core_ids=list(range(NCORES)))
            ts.append(time.perf_counter() - t0)
        walls[reps] = min(ts)
        print(f"reps={reps}: wall min={walls[reps]*1e3:.2f} ms  all={[f'{t*1e3:.1f}' for t in ts]}")
    r0, r1 = reps_pair
    ns = (walls[r1] - walls[r0]) / (r1 - r0) * 1e9
    print(f"per-body device time: {ns:.0f} ns")
    return ns


# revision 19
# speedup vs baseline: 1.4664x; 1.0104x over previous
"""Cross-modal attention TRN2 kernel (fp16 pipeline).

Problem: B=4, N=2048, IN_DIM=DIM=1024, HEADS=8, D_HEAD=128, scale=DIM**-0.5.
  q = x_a @ W_q.T ; k,v = split(x_b @ W_kv.T) ; per-head softmax(q k^T/32) v ;
  out = merge_heads @ W_out.T + b_out

Sharding over 8 cores: core c -> batch b=c//2, head-half hh=c%2 (4 heads,
512 of DIM).  W_q/W_kv column-sharded, W_out row-sharded (Megatron); each
core emits a partial output projection y_cT = (W_out[:, slice] @ O_half)
of shape [DIM, N]; host sums the two head-half partials per batch, adds
b_out, transposes back.

Device layout: everything transposed ([feature, token]) so all matmuls
contract over the partition dim.  All matmul operands are fp16 (host
casts x and W), PSUM accumulation f32, y written back fp16; simulated
end-to-end rel-err vs the f32 reference ~2.4e-3 (tolerance 2e-2).

  phase 1: K^T/V from x_b blocks (shared SBUF block), Q^T from x_a;
           512-wide moving blocks, one [128,512] psum per (dt|jt) with
           8 contraction matmuls.  K/Q psum->SBUF copies on DVE, V on ACT.
  phase 2: per (1024-i-block, head): for each of 16 j-tiles: dots^T
           (2 matmuls), exp on ACT (no max subtraction; |s*scale| < ~1),
           and the softmax denominator accumulated OFF the PE: DVE sums
           even j-tiles into accV, GpSimd odd into accG (elementwise adds
           of the exp tiles).  PV trails by LAG j-tiles in a second psum.
           Epilogue: 4 narrow reduce-matmuls (all-ones stationary) give
           den as a [128,1024] psum (rows identical), DVE
           reciprocal_approx_fast -> rc, one fused DVE multiply
           osl = po * rc drains PV psum.
  phase 3: y^T = WoT.T @ O^T (512-wide), staging copies alternate
           DVE/ACT, DMA -> DRAM f32.
"""

import numpy as np

B, N, IN_DIM, DIM, HEADS = 4, 2048, 1024, 1024, 8
D_HEAD = DIM // HEADS          # 128
SCALE = DIM ** -0.5            # 1/32
NCORES = 8
HH = HEADS // 2                # 4 heads per core
DVC = HH * D_HEAD              # 512 dv per core
P = 128
KT = IN_DIM // P               # 8 contraction tiles
NJT = N // P                   # 16 j tiles
NIB = N // 512                 # 4 i-blocks of 512
IB2 = N // 1024                # 2 i-blocks of 1024

_TRACE = False
REPS = 1
LAST_EXEC_NS = None
LAST_RESULTS = None


def _build_nc(reps=1):
    import concourse.tile as tile
    from concourse import bacc, mybir

    f32 = mybir.dt.float32
    f16 = mybir.dt.float16
    f8 = mybir.dt.float8e4
    DR = mybir.MatmulPerfMode.DoubleRow
    Exp = mybir.ActivationFunctionType.Exp

    nc = bacc.Bacc("TRN2", debug=False, num_devices=NCORES)

    xaT = nc.dram_tensor("xaT", [IN_DIM, N], f8, kind="ExternalInput").ap()
    xbT = nc.dram_tensor("xbT", [IN_DIM, N], f16, kind="ExternalInput").ap()
    wqT = nc.dram_tensor("wqT", [IN_DIM, DVC], f8, kind="ExternalInput").ap()
    wkT = nc.dram_tensor("wkT", [IN_DIM, DVC], f16, kind="ExternalInput").ap()
    wvT = nc.dram_tensor("wvT", [IN_DIM, DVC], f16, kind="ExternalInput").ap()
    woT = nc.dram_tensor("woT", [DVC, DIM], f16, kind="ExternalInput").ap()
    ones_d = nc.dram_tensor("ones", [P, P], f16, kind="ExternalInput").ap()
    yT = nc.dram_tensor("yT", [DIM, N], f16, kind="ExternalOutput").ap()

    BW = 512                   # moving-block width (max moving free dim)
    NB = N // BW               # 4 blocks

    with tile.TileContext(nc) as tc:
      for _rep in range(reps):
        with tc.tile_pool(name="persist", bufs=1) as persist:
            qT_sb = persist.tile([P, HH, N], f16)      # [d%128, head, i]
            kT_sb = persist.tile([P, HH, N], f16)      # [d%128, head, j]
            v_sb = persist.tile([P, NJT, DVC], f16)    # [j%128, jt, dv]
            oT_ts = [[persist.tile([P, 1024], f16, tag=f"o{h}_{bb}",
                                   name=f"o{h}_{bb}")
                      for bb in range(IB2)] for h in range(HH)]
            ones_sb = persist.tile([P, P], f16)
            wo_sb = persist.tile([P, HH, DIM], f16)
            nc.sync.dma_start(out=ones_sb, in_=ones_d)

            # ---------------- phase 1: projections ----------------
            with tc.tile_pool(name="wpool", bufs=1) as wpool, \
                 tc.tile_pool(name="xblk", bufs=3) as xblk, \
                 tc.tile_pool(name="psum1", bufs=6, space="PSUM") as psum1:
                wk_sb = wpool.tile([P, KT, DVC], f16, name="wk")
                wv_sb = wpool.tile([P, KT, DVC], f16, name="wv")
                wq_sb = wpool.tile([P, KT, DVC], f8, name="wq")
                # single in-order sync DGE queue, sequenced by first use:
                # the dt=0 slice of wk (256KB) unblocks the very first
                # matmul, then the first x block, then the rest
                nc.sync.dma_start(
                    out=wk_sb[:, :, 0:P],
                    in_=wkT[:, 0:P].rearrange("(kt p) d -> p kt d", p=P))

                for jb in range(NB):
                    xb_blk = xblk.tile([P, KT, BW], f16, tag="xblk")
                    if jb == 0:
                        # split the first block so its kt=0..3 half (the
                        # first psum chain's operands) lands sooner
                        for hkt in range(2):
                            nc.sync.dma_start(
                                out=xb_blk[:, hkt * KT // 2:(hkt + 1) * KT // 2, :],
                                in_=xbT[hkt * IN_DIM // 2:(hkt + 1) * IN_DIM // 2,
                                        0:BW]
                                .rearrange("(kt p) i -> p kt i", p=P))
                    else:
                        nc.sync.dma_start(
                            out=xb_blk,
                            in_=xbT[:, jb * BW:(jb + 1) * BW]
                            .rearrange("(kt p) i -> p kt i", p=P))
                    if jb == 0:
                        nc.sync.dma_start(
                            out=wk_sb[:, :, P:DVC],
                            in_=wkT[:, P:DVC]
                            .rearrange("(kt p) d -> p kt d", p=P))
                        nc.sync.dma_start(
                            out=wv_sb,
                            in_=wvT.rearrange("(kt p) d -> p kt d", p=P))
                    elif jb == 1:
                        nc.sync.dma_start(
                            out=wq_sb,
                            in_=wqT.rearrange("(kt p) d -> p kt d", p=P))
                    for dt in range(HH):
                        ps = psum1.tile([P, BW], f32, tag="ps1")
                        for kt in range(KT):
                            nc.tensor.matmul(
                                ps,
                                wk_sb[:, kt, dt * P:(dt + 1) * P],
                                xb_blk[:, kt, :],
                                start=(kt == 0), stop=(kt == KT - 1))
                        nc.vector.tensor_copy(
                            kT_sb[:, dt, jb * BW:(jb + 1) * BW], ps)
                    for j2 in range(BW // P):
                        jt = jb * (BW // P) + j2
                        ps = psum1.tile([P, DVC], f32, tag="ps1")
                        for kt in range(KT):
                            nc.tensor.matmul(
                                ps,
                                xb_blk[:, kt, j2 * P:(j2 + 1) * P],
                                wv_sb[:, kt, :],
                                start=(kt == 0), stop=(kt == KT - 1))
                        nc.scalar.copy(v_sb[:, jt, :], ps)

                for ib in range(NB):
                    xa_blk = xblk.tile([P, KT, BW], f8, tag="xa8", name="xa8")
                    nc.sync.dma_start(
                        out=xa_blk,
                        in_=xaT[:, ib * BW:(ib + 1) * BW]
                        .rearrange("(kt p) i -> p kt i", p=P))
                    if ib == 0:
                        # wo prefetch: needed only in phase 3
                        nc.sync.dma_start(
                            out=wo_sb,
                            in_=woT.rearrange("(dt p) e -> p dt e", p=P))
                    for dt in range(HH):
                        ps = psum1.tile([P, BW], f32, tag="ps1")
                        for kp in range(KT // 2):
                            # fp8 DoubleRow: two 128-deep k-tiles per pass,
                            # 2x PE rate; W_q is host-scaled by 16 so the
                            # psum holds 16*q (folded into the exp scale)
                            nc.tensor.matmul(
                                ps,
                                wq_sb[:, 2 * kp:2 * kp + 2, dt * P:(dt + 1) * P],
                                xa_blk[:, 2 * kp:2 * kp + 2, :],
                                start=(kp == 0), stop=(kp == KT // 2 - 1),
                                perf_mode=DR)
                        nc.vector.tensor_copy(
                            qT_sb[:, dt, ib * BW:(ib + 1) * BW], ps)

            # ---------------- phase 2: attention + phase 3 ----------------
            with tc.tile_pool(name="expp", bufs=8) as expp, \
                 tc.tile_pool(name="accp", bufs=2) as accp, \
                 tc.tile_pool(name="rcp", bufs=2) as rcp, \
                 tc.tile_pool(name="ystage", bufs=4) as ystage, \
                 tc.tile_pool(name="dotsp", bufs=2, space="PSUM") as dotsp, \
                 tc.tile_pool(name="avp", bufs=2, space="PSUM") as avp:
                LAG = 3   # PV trails dots/exp by 3 j-tiles so the PE
                          # never waits on the ACT exp of the current tile
                PE_JTS = (14, 15)      # denominator tiles summed by direct
                                       # ones-matmuls at the epilogue; the
                                       # rest go to DVE (even) / GpSimd (odd)

                def emit_epilogue(po, accV, accG, pe_ets, osl):
                    # partition-reduce accumulators + the PE_JTS exp tiles
                    # with an all-ones stationary: pd rows all = the denom.
                    # pd borrows a dots-ring psum slot (released after rc).
                    pd = dotsp.tile([P, 1024], f32, tag="ps")
                    movs = [accV, accG] + pe_ets
                    for hf in range(2):
                        sl = slice(hf * 512, (hf + 1) * 512)
                        for mi, mv in enumerate(movs):
                            nc.tensor.matmul(pd[:, sl], ones_sb, mv[:, sl],
                                             start=(mi == 0),
                                             stop=(mi == len(movs) - 1))
                    rc = rcp.tile([P, 1024], f32, tag="rc")
                    nc.vector.reciprocal_approx_fast(rc, pd)
                    nc.vector.tensor_mul(osl, po, rc)

                def emit_proj_chunk(ibq):
                    # output projection for one 512-wide i-slice; psum from
                    # the dots ring, staging copies split DVE/ACT
                    bb, half = divmod(ibq, 2)
                    for e8 in range(DIM // P):
                        ps = dotsp.tile([P, 512], f32, tag="ps")
                        for dt in range(HH):
                            nc.tensor.matmul(
                                ps,
                                wo_sb[:, dt, e8 * P:(e8 + 1) * P],
                                oT_ts[dt][bb][:, half * 512:(half + 1) * 512],
                                start=(dt == 0), stop=(dt == HH - 1))
                        ys = ystage.tile([P, 512], f16, tag="ys")
                        if e8 % 2 == 0:
                            nc.vector.tensor_copy(ys, ps)
                        else:
                            nc.scalar.copy(ys, ps)
                        nc.sync.dma_start(
                            out=yT[e8 * P:(e8 + 1) * P,
                                   ibq * 512:(ibq + 1) * 512],
                            in_=ys)

                pending = None
                blocks = [(ib, h) for ib in range(IB2) for h in range(HH)]
                for bi, (ib, h) in enumerate(blocks):
                    i0 = ib * 1024
                    po = avp.tile([P, 1024], f32, tag="po")
                    accV = accp.tile([P, 1024], f16, tag="accV")
                    accG = accp.tile([P, 1024], f16, tag="accG")
                    ets = {}
                    pe_ets = []
                    for jt in range(NJT + LAG):
                        if jt < NJT:
                            ps = dotsp.tile([P, 1024], f32, tag="ps")
                            k_l = kT_sb[:, h, jt * P:(jt + 1) * P]
                            for hf in range(2):
                                nc.tensor.matmul(
                                    ps[:, hf * 512:(hf + 1) * 512],
                                    k_l,
                                    qT_sb[:, h,
                                          i0 + hf * 512:i0 + (hf + 1) * 512],
                                    start=True, stop=True)
                            et = expp.tile([P, 1024], f16, tag="exp")
                            nc.scalar.activation(et, ps, Exp, scale=SCALE / 16.0)
                            ets[jt] = et
                            # softmax denominator partials
                            if jt in PE_JTS:
                                pe_ets.append(et)
                            elif jt == 0:
                                nc.vector.tensor_copy(accV, et)
                            elif jt == 1:
                                nc.vector.tensor_copy(accG, et)
                            elif (jt % 2 == 0) != (jt in (12, 13)):
                                # DVE: evens 2-10 plus 13; GpSimd: odds 3-11
                                # plus 12 (6 adds each)
                                nc.vector.tensor_add(accV, accV, et)
                            else:
                                nc.gpsimd.tensor_add(accG, accG, et)
                        if jt == 3 and pending is not None:
                            # previous block's epilogue, deferred so its
                            # reduce-matmuls never stall the PE pipeline
                            pending()
                            pending = None
                        if jt >= LAG:
                            jd = jt - LAG
                            et = ets.pop(jd)
                            v_l = v_sb[:, jd, h * P:(h + 1) * P]
                            for hf in range(2):
                                sl = slice(hf * 512, (hf + 1) * 512)
                                nc.tensor.matmul(
                                    po[:, sl], v_l, et[:, sl],
                                    start=(jd == 0), stop=(jd == NJT - 1))
                    import functools
                    pending = functools.partial(
                        emit_epilogue, po, accV, accG, pe_ets, oT_ts[h][ib])
                    # first half of the output projection interleaves at the
                    # block boundaries after the ib=0 oT tiles are complete
                    if bi == 4:
                        emit_proj_chunk(0)
                    elif bi == 5:
                        emit_proj_chunk(1)
                pending()
                emit_proj_chunk(2)
                emit_proj_chunk(3)

    nc.compile()
    return nc


_nc_by_reps = {}


def _get_nc(reps=1):
    if reps not in _nc_by_reps:
        _nc_by_reps[reps] = _build_nc(reps)
    return _nc_by_reps[reps]


def _make_in_maps(x_a, x_b, W_q, W_kv, W_out):
    import ml_dtypes
    f16 = np.float16
    f8 = ml_dtypes.float8_e4m3

    xaT = [np.ascontiguousarray(x_a[b].T).astype(f8) for b in range(B)]
    xbT = [np.ascontiguousarray(x_b[b].T).astype(f16) for b in range(B)]
    in_maps = []
    for c in range(NCORES):
        b, hh = divmod(c, 2)
        hs = hh * DVC
        in_maps.append({
            "xaT": xaT[b],
            "xbT": xbT[b],
            "wqT": np.ascontiguousarray(16.0 * W_q[hs:hs + DVC].T).astype(f8),
            "wkT": np.ascontiguousarray(W_kv[hs:hs + DVC].T).astype(f16),
            "wvT": np.ascontiguousarray(
                W_kv[DIM + hs:DIM + hs + DVC].T).astype(f16),
            "woT": np.ascontiguousarray(W_out[:, hs:hs + DVC].T).astype(f16),
            "ones": np.ones((P, P), dtype=f16),
        })
    return in_maps


def kernel(x_a, x_b, W_q, W_kv, W_out, b_out):
    global LAST_EXEC_NS, LAST_RESULTS
    from concourse import bass_utils

    x_a = np.asarray(x_a, dtype=np.float32)
    x_b = np.asarray(x_b, dtype=np.float32)
    W_q = np.asarray(W_q, dtype=np.float32)
    W_kv = np.asarray(W_kv, dtype=np.float32)
    W_out = np.asarray(W_out, dtype=np.float32)
    b_out = np.asarray(b_out, dtype=np.float32)

    nc = _get_nc(REPS)
    in_maps = _make_in_maps(x_a, x_b, W_q, W_kv, W_out)

    res = bass_utils.run_bass_kernel_spmd(
        nc, in_maps, core_ids=list(range(NCORES)), trace=_TRACE)
    LAST_EXEC_NS = res.exec_time_ns
    LAST_RESULTS = res

    out = np.empty((B, N, DIM), dtype=np.float32)
    for b in range(B):
        acc = (res.results[2 * b]["yT"].astype(np.float32)
               + res.results[2 * b + 1]["yT"].astype(np.float32))
        out[b] = acc.T + b_out
    return out


def bench(inputs, reps_pair=(1, 9), iters=5):
    """Measure on-device time per kernel body via rep-delta wall timing."""
    import time
    from concourse import bass_utils
    ins = {k: np.asarray(v, dtype=np.float32) for k, v in inputs.items()
           if k != "b_out"}
    in_maps = _make_in_maps(ins["x_a"], ins["x_b"], ins["W_q"], ins["W_kv"],
                            ins["W_out"])
    walls = {}
    for reps in reps_pair:
        nc = _get_nc(reps)
        # warm-up (compile+cache)
        bass_utils.run_bass_kernel_spmd(nc, in_maps, core_ids=list(range(NCORES)))
        ts = []
        for _ in range(iters):
            t0 = time.perf_counter()
            bass_utils.run_bass_kernel_spmd(nc, in_maps,
                                            core_ids=list(range(NCORES)))
            ts.append(time.perf_counter() - t0)
        walls[reps] = min(ts)
        print(f"reps={reps}: wall min={walls[reps]*1e3:.2f} ms  all={[f'{t*1e3:.1f}' for t in ts]}")
    r0, r1 = reps_pair
    ns = (walls[r1] - walls[r0]) / (r1 - r0) * 1e9
    print(f"per-body device time: {ns:.0f} ns")
    return ns


# revision 21
# speedup vs baseline: 1.4679x; 1.0011x over previous
"""Cross-modal attention TRN2 kernel (fp16 pipeline).

Problem: B=4, N=2048, IN_DIM=DIM=1024, HEADS=8, D_HEAD=128, scale=DIM**-0.5.
  q = x_a @ W_q.T ; k,v = split(x_b @ W_kv.T) ; per-head softmax(q k^T/32) v ;
  out = merge_heads @ W_out.T + b_out

Sharding over 8 cores: core c -> batch b=c//2, head-half hh=c%2 (4 heads,
512 of DIM).  W_q/W_kv column-sharded, W_out row-sharded (Megatron); each
core emits a partial output projection y_cT = (W_out[:, slice] @ O_half)
of shape [DIM, N]; host sums the two head-half partials per batch, adds
b_out, transposes back.

Device layout: everything transposed ([feature, token]) so all matmuls
contract over the partition dim.  All matmul operands are fp16 (host
casts x and W), PSUM accumulation f32, y written back fp16; simulated
end-to-end rel-err vs the f32 reference ~2.4e-3 (tolerance 2e-2).

  phase 1: K^T/V from x_b blocks (shared SBUF block), Q^T from x_a;
           512-wide moving blocks, one [128,512] psum per (dt|jt) with
           8 contraction matmuls.  K/Q psum->SBUF copies on DVE, V on ACT.
  phase 2: per (1024-i-block, head): for each of 16 j-tiles: dots^T
           (2 matmuls), exp on ACT (no max subtraction; |s*scale| < ~1),
           and the softmax denominator accumulated OFF the PE: DVE sums
           even j-tiles into accV, GpSimd odd into accG (elementwise adds
           of the exp tiles).  PV trails by LAG j-tiles in a second psum.
           Epilogue: 4 narrow reduce-matmuls (all-ones stationary) give
           den as a [128,1024] psum (rows identical), DVE
           reciprocal_approx_fast -> rc, one fused DVE multiply
           osl = po * rc drains PV psum.
  phase 3: y^T = WoT.T @ O^T (512-wide), staging copies alternate
           DVE/ACT, DMA -> DRAM f32.
"""

import functools

import numpy as np

B, N, IN_DIM, DIM, HEADS = 4, 2048, 1024, 1024, 8
D_HEAD = DIM // HEADS          # 128
SCALE = DIM ** -0.5            # 1/32
NCORES = 8
HH = HEADS // 2                # 4 heads per core
DVC = HH * D_HEAD              # 512 dv per core
P = 128
KT = IN_DIM // P               # 8 contraction tiles
NJT = N // P                   # 16 j tiles
NIB = N // 512                 # 4 i-blocks of 512
IB2 = N // 1024                # 2 i-blocks of 1024

_TRACE = False
REPS = 1
LAST_EXEC_NS = None
LAST_RESULTS = None


def _build_nc(reps=1):
    import concourse.tile as tile
    from concourse import bacc, mybir

    f32 = mybir.dt.float32
    f16 = mybir.dt.float16
    f8 = mybir.dt.float8e4
    DR = mybir.MatmulPerfMode.DoubleRow
    Exp = mybir.ActivationFunctionType.Exp

    nc = bacc.Bacc("TRN2", debug=False, num_devices=NCORES)

    xaT = nc.dram_tensor("xaT", [IN_DIM, N], f8, kind="ExternalInput").ap()
    xbT = nc.dram_tensor("xbT", [IN_DIM, N], f16, kind="ExternalInput").ap()
    wqT = nc.dram_tensor("wqT", [IN_DIM, DVC], f8, kind="ExternalInput").ap()
    wkT = nc.dram_tensor("wkT", [IN_DIM, DVC], f16, kind="ExternalInput").ap()
    wvT = nc.dram_tensor("wvT", [IN_DIM, DVC], f16, kind="ExternalInput").ap()
    woT = nc.dram_tensor("woT", [DVC, DIM], f16, kind="ExternalInput").ap()
    ones_d = nc.dram_tensor("ones", [P, P], f16, kind="ExternalInput").ap()
    yT = nc.dram_tensor("yT", [DIM, N], f16, kind="ExternalOutput").ap()

    BW = 512                   # moving-block width (max moving free dim)
    NB = N // BW               # 4 blocks

    with tile.TileContext(nc) as tc:
      for _rep in range(reps):
        with tc.tile_pool(name="persist", bufs=1) as persist:
            qT_sb = persist.tile([P, HH, N], f16)      # [d%128, head, i]
            kT_sb = persist.tile([P, HH, N], f16)      # [d%128, head, j]
            v_sb = persist.tile([P, NJT, DVC], f16)    # [j%128, jt, dv]
            oT_ts = [[persist.tile([P, 1024], f16, tag=f"o{h}_{bb}",
                                   name=f"o{h}_{bb}")
                      for bb in range(IB2)] for h in range(HH)]
            ones_sb = persist.tile([P, P], f16)
            wo_sb = persist.tile([P, HH, DIM], f16)
            nc.sync.dma_start(out=ones_sb, in_=ones_d)

            # ---------------- phase 1: projections ----------------
            with tc.tile_pool(name="wpool", bufs=1) as wpool, \
                 tc.tile_pool(name="xblk", bufs=3) as xblk, \
                 tc.tile_pool(name="psum1", bufs=6, space="PSUM") as psum1:
                wk_sb = wpool.tile([P, KT, DVC], f16, name="wk")
                wv_sb = wpool.tile([P, KT, DVC], f16, name="wv")
                wq_sb = wpool.tile([P, KT, DVC], f8, name="wq")
                # block 0 of x_b as two standalone half-tiles: the kt=0..3
                # half rides the (otherwise idle) gpsimd SWDGE queue in
                # parallel with wk's dt=0 slice on the sync queue, so the
                # first matmul's operands land ~3us sooner
                xb0a = wpool.tile([P, KT // 2, BW], f16, name="xb0a")
                xb0b = wpool.tile([P, KT // 2, BW], f16, name="xb0b")
                nc.gpsimd.dma_start(
                    out=xb0a,
                    in_=xbT[0:IN_DIM // 2, 0:BW]
                    .rearrange("(kt p) i -> p kt i", p=P))
                nc.sync.dma_start(
                    out=wk_sb[:, :, 0:P],
                    in_=wkT[:, 0:P].rearrange("(kt p) d -> p kt d", p=P))
                nc.sync.dma_start(
                    out=xb0b,
                    in_=xbT[IN_DIM // 2:IN_DIM, 0:BW]
                    .rearrange("(kt p) i -> p kt i", p=P))

                for jb in range(NB):
                    if jb == 0:
                        def xb_sl(kt, lo, hi):
                            t = xb0a if kt < KT // 2 else xb0b
                            return t[:, kt % (KT // 2), lo:hi]
                        nc.sync.dma_start(
                            out=wk_sb[:, :, P:DVC],
                            in_=wkT[:, P:DVC]
                            .rearrange("(kt p) d -> p kt d", p=P))
                        nc.sync.dma_start(
                            out=wv_sb,
                            in_=wvT.rearrange("(kt p) d -> p kt d", p=P))
                    else:
                        xb_blk = xblk.tile([P, KT, BW], f16, tag="xblk")
                        nc.sync.dma_start(
                            out=xb_blk,
                            in_=xbT[:, jb * BW:(jb + 1) * BW]
                            .rearrange("(kt p) i -> p kt i", p=P))

                        def xb_sl(kt, lo, hi, _t=xb_blk):
                            return _t[:, kt, lo:hi]
                        if jb == 1:
                            nc.sync.dma_start(
                                out=wq_sb,
                                in_=wqT.rearrange("(kt p) d -> p kt d", p=P))
                    for dt in range(HH):
                        ps = psum1.tile([P, BW], f32, tag="ps1")
                        for kt in range(KT):
                            nc.tensor.matmul(
                                ps,
                                wk_sb[:, kt, dt * P:(dt + 1) * P],
                                xb_sl(kt, 0, BW),
                                start=(kt == 0), stop=(kt == KT - 1))
                        nc.vector.tensor_copy(
                            kT_sb[:, dt, jb * BW:(jb + 1) * BW], ps)
                    for j2 in range(BW // P):
                        jt = jb * (BW // P) + j2
                        ps = psum1.tile([P, DVC], f32, tag="ps1")
                        for kt in range(KT):
                            nc.tensor.matmul(
                                ps,
                                xb_sl(kt, j2 * P, (j2 + 1) * P),
                                wv_sb[:, kt, :],
                                start=(kt == 0), stop=(kt == KT - 1))
                        nc.scalar.copy(v_sb[:, jt, :], ps)

                for ib in range(NB):
                    xa_blk = xblk.tile([P, KT, BW], f8, tag="xa8", name="xa8")
                    nc.sync.dma_start(
                        out=xa_blk,
                        in_=xaT[:, ib * BW:(ib + 1) * BW]
                        .rearrange("(kt p) i -> p kt i", p=P))
                    if ib == 0:
                        # wo prefetch: needed only in phase 3
                        nc.sync.dma_start(
                            out=wo_sb,
                            in_=woT.rearrange("(dt p) e -> p dt e", p=P))
                    for dt in range(HH):
                        ps = psum1.tile([P, BW], f32, tag="ps1")
                        for kp in range(KT // 2):
                            # fp8 DoubleRow: two 128-deep k-tiles per pass,
                            # 2x PE rate; W_q is host-scaled by 16 so the
                            # psum holds 16*q (folded into the exp scale)
                            nc.tensor.matmul(
                                ps,
                                wq_sb[:, 2 * kp:2 * kp + 2, dt * P:(dt + 1) * P],
                                xa_blk[:, 2 * kp:2 * kp + 2, :],
                                start=(kp == 0), stop=(kp == KT // 2 - 1),
                                perf_mode=DR)
                        nc.vector.tensor_copy(
                            qT_sb[:, dt, ib * BW:(ib + 1) * BW], ps)

            # ---------------- phase 2: attention + phase 3 ----------------
            with tc.tile_pool(name="expp", bufs=8) as expp, \
                 tc.tile_pool(name="accp", bufs=2) as accp, \
                 tc.tile_pool(name="rcp", bufs=2) as rcp, \
                 tc.tile_pool(name="ystage", bufs=4) as ystage, \
                 tc.tile_pool(name="dotsp", bufs=2, space="PSUM") as dotsp, \
                 tc.tile_pool(name="avp", bufs=2, space="PSUM") as avp:
                LAG = 3   # PV trails dots/exp by 3 j-tiles so the PE
                          # never waits on the ACT exp of the current tile
                PE_JTS = (14, 15)      # denominator tiles summed by direct
                                       # ones-matmuls at the epilogue; the
                                       # rest go to DVE (even) / GpSimd (odd)

                def emit_epilogue(po, accV, accG, pe_ets, osl):
                    # partition-reduce accumulators + the PE_JTS exp tiles
                    # with an all-ones stationary: pd rows all = the denom.
                    # pd borrows a dots-ring psum slot (released after rc).
                    pd = dotsp.tile([P, 1024], f32, tag="ps")
                    movs = [accV, accG] + pe_ets
                    for hf in range(2):
                        sl = slice(hf * 512, (hf + 1) * 512)
                        for mi, mv in enumerate(movs):
                            nc.tensor.matmul(pd[:, sl], ones_sb, mv[:, sl],
                                             start=(mi == 0),
                                             stop=(mi == len(movs) - 1))
                    rc = rcp.tile([P, 1024], f32, tag="rc")
                    nc.vector.reciprocal_approx_fast(rc, pd)
                    nc.vector.tensor_mul(osl, po, rc)

                def emit_proj_chunk(ibq):
                    # output projection for one 512-wide i-slice; psum from
                    # the dots ring, staging copies split DVE/ACT
                    bb, half = divmod(ibq, 2)
                    for e8 in range(DIM // P):
                        ps = dotsp.tile([P, 512], f32, tag="ps")
                        for dt in range(HH):
                            nc.tensor.matmul(
                                ps,
                                wo_sb[:, dt, e8 * P:(e8 + 1) * P],
                                oT_ts[dt][bb][:, half * 512:(half + 1) * 512],
                                start=(dt == 0), stop=(dt == HH - 1))
                        ys = ystage.tile([P, 512], f16, tag="ys")
                        if e8 % 2 == 0:
                            nc.vector.tensor_copy(ys, ps)
                        else:
                            nc.scalar.copy(ys, ps)
                        nc.sync.dma_start(
                            out=yT[e8 * P:(e8 + 1) * P,
                                   ibq * 512:(ibq + 1) * 512],
                            in_=ys)

                pending = None
                blocks = [(ib, h) for ib in range(IB2) for h in range(HH)]
                for bi, (ib, h) in enumerate(blocks):
                    i0 = ib * 1024
                    po = avp.tile([P, 1024], f32, tag="po")
                    accV = accp.tile([P, 1024], f16, tag="accV")
                    accG = accp.tile([P, 1024], f16, tag="accG")
                    ets = {}
                    pe_ets = []
                    for jt in range(NJT + LAG):
                        if jt < NJT:
                            ps = dotsp.tile([P, 1024], f32, tag="ps")
                            k_l = kT_sb[:, h, jt * P:(jt + 1) * P]
                            for hf in range(2):
                                nc.tensor.matmul(
                                    ps[:, hf * 512:(hf + 1) * 512],
                                    k_l,
                                    qT_sb[:, h,
                                          i0 + hf * 512:i0 + (hf + 1) * 512],
                                    start=True, stop=True)
                            et = expp.tile([P, 1024], f16, tag="exp")
                            nc.scalar.activation(et, ps, Exp, scale=SCALE / 16.0)
                            ets[jt] = et
                            # softmax denominator partials
                            if jt in PE_JTS:
                                pe_ets.append(et)
                            elif jt == 0:
                                nc.vector.tensor_copy(accV, et)
                            elif jt == 1:
                                nc.vector.tensor_copy(accG, et)
                            elif (jt % 2 == 0) != (jt in (12, 13)):
                                # DVE: evens 2-10 plus 13; GpSimd: odds 3-11
                                # plus 12 (6 adds each)
                                nc.vector.tensor_add(accV, accV, et)
                            else:
                                nc.gpsimd.tensor_add(accG, accG, et)
                        if jt == 3 and pending is not None:
                            # previous block's epilogue, deferred so its
                            # reduce-matmuls never stall the PE pipeline
                            pending()
                            pending = None
                        if jt >= LAG:
                            jd = jt - LAG
                            et = ets.pop(jd)
                            v_l = v_sb[:, jd, h * P:(h + 1) * P]
                            for hf in range(2):
                                sl = slice(hf * 512, (hf + 1) * 512)
                                nc.tensor.matmul(
                                    po[:, sl], v_l, et[:, sl],
                                    start=(jd == 0), stop=(jd == NJT - 1))
                    pending = functools.partial(
                        emit_epilogue, po, accV, accG, pe_ets, oT_ts[h][ib])
                    # first half of the output projection interleaves at the
                    # block boundaries after the ib=0 oT tiles are complete
                    if bi == 4:
                        emit_proj_chunk(0)
                    elif bi == 5:
                        emit_proj_chunk(1)
                pending()
                emit_proj_chunk(2)
                emit_proj_chunk(3)

    nc.compile()
    return nc


_nc_by_reps = {}


def _get_nc(reps=1):
    if reps not in _nc_by_reps:
        _nc_by_reps[reps] = _build_nc(reps)
    return _nc_by_reps[reps]


def _make_in_maps(x_a, x_b, W_q, W_kv, W_out):
    import ml_dtypes
    f16 = np.float16
    f8 = ml_dtypes.float8_e4m3

    xaT = [np.ascontiguousarray(x_a[b].T).astype(f8) for b in range(B)]
    xbT = [np.ascontiguousarray(x_b[b].T).astype(f16) for b in range(B)]
    in_maps = []
    for c in range(NCORES):
        b, hh = divmod(c, 2)
        hs = hh * DVC
        in_maps.append({
            "xaT": xaT[b],
            "xbT": xbT[b],
            "wqT": np.ascontiguousarray(16.0 * W_q[hs:hs + DVC].T).astype(f8),
            "wkT": np.ascontiguousarray(W_kv[hs:hs + DVC].T).astype(f16),
            "wvT": np.ascontiguousarray(
                W_kv[DIM + hs:DIM + hs + DVC].T).astype(f16),
            "woT": np.ascontiguousarray(W_out[:, hs:hs + DVC].T).astype(f16),
            "ones": np.ones((P, P), dtype=f16),
        })
    return in_maps


def kernel(x_a, x_b, W_q, W_kv, W_out, b_out):
    global LAST_EXEC_NS, LAST_RESULTS
    from concourse import bass_utils

    x_a = np.asarray(x_a, dtype=np.float32)
    x_b = np.asarray(x_b, dtype=np.float32)
    W_q = np.asarray(W_q, dtype=np.float32)
    W_kv = np.asarray(W_kv, dtype=np.float32)
    W_out = np.asarray(W_out, dtype=np.float32)
    b_out = np.asarray(b_out, dtype=np.float32)

    nc = _get_nc(REPS)
    in_maps = _make_in_maps(x_a, x_b, W_q, W_kv, W_out)

    res = bass_utils.run_bass_kernel_spmd(
        nc, in_maps, core_ids=list(range(NCORES)), trace=_TRACE)
    LAST_EXEC_NS = res.exec_time_ns
    LAST_RESULTS = res

    out = np.empty((B, N, DIM), dtype=np.float32)
    for b in range(B):
        acc = (res.results[2 * b]["yT"].astype(np.float32)
               + res.results[2 * b + 1]["yT"].astype(np.float32))
        out[b] = acc.T + b_out
    return out


def bench(inputs, reps_pair=(1, 9), iters=5):
    """Measure on-device time per kernel body via rep-delta wall timing."""
    import time
    from concourse import bass_utils
    ins = {k: np.asarray(v, dtype=np.float32) for k, v in inputs.items()
           if k != "b_out"}
    in_maps = _make_in_maps(ins["x_a"], ins["x_b"], ins["W_q"], ins["W_kv"],
                            ins["W_out"])
    walls = {}
    for reps in reps_pair:
        nc = _get_nc(reps)
        # warm-up (compile+cache)
        bass_utils.run_bass_kernel_spmd(nc, in_maps, core_ids=list(range(NCORES)))
        ts = []
        for _ in range(iters):
            t0 = time.perf_counter()
            bass_utils.run_bass_kernel_spmd(nc, in_maps,
                                            core_ids=list(range(NCORES)))
            ts.append(time.perf_counter() - t0)
        walls[reps] = min(ts)
        print(f"reps={reps}: wall min={walls[reps]*1e3:.2f} ms  all={[f'{t*1e3:.1f}' for t in ts]}")
    r0, r1 = reps_pair
    ns = (walls[r1] - walls[r0]) / (r1 - r0) * 1e9
    print(f"per-body device time: {ns:.0f} ns")
    return ns
